# revision 32
# baseline (speedup 1.0000x reference)
"""2-layer GAT (DGL GATConv-style) on 8 Trainium2 NeuronCores.

Strategy (dst-sharded message passing):
  - Nodes are partitioned across 8 cores round-robin by global in-degree
    rank; core c owns ~6250 dsts and all edges pointing into them.
  - Per-core, own dsts are degree-sorted and tiled 128-at-a-time.  The HBM
    z-table rows are laid out in exactly this (core, tile, slot) order, so
    layer-2 tile results can be written back CONTIGUOUSLY (no scatter), and
    both layers share one set of gather indices.
  - Layer-1 projection z = x @ W1 is computed replicated on every core and
    written to an HBM gather table of 512B bf16 rows.  The per-edge source
    attention term el1[src] is precomputed on the HOST (x @ W1 @ a_l is
    cheap) and staged directly in per-slot layout, so gather rows carry
    only z.
  - Per tile, source rows are fetched with the dma_gather custom DMA
    (int16 indices -> the table is addressed via two overlapping windows
    A=[0,32768) and B=[ROWS-32768,ROWS)).  Edge softmax runs on the
    gathered tile: w = exp(lrelu(el_src + er_dst)), denominators via ACT
    accum, weighting + segment sum on the vector engine (in-place multiply
    + binary-tree folds over the slot axis).
  - Layer-2 projection z2aug = h @ [W2|W2@AL2|W2@AR2] is computed from the
    layer-1 tiles (PE transpose), written contiguously into a per-core
    shard, exchanged with an AllGather collective, then the same
    gather/softmax machinery produces the output.
"""
import sys

sys.path.insert(0, "/opt/trn_rl_repo")

import numpy as np

import concourse.bass as bass
import concourse.mybir as mybir
import concourse.tile as tile_mod
from concourse import library_config
from concourse.library_overlay import lower_extended_insts
from concourse.tile import TileContext
from concourse.bass_utils import run_bass_kernel_spmd

F32 = mybir.dt.float32
BF16 = mybir.dt.bfloat16
I16 = mybir.dt.int16
AF = mybir.ActivationFunctionType
ALU = mybir.AluOpType

NEG_SLOPE = 0.2
SENT_EL = -1.0e30


# ---------------------------------------------------------------------------
# Workaround: this walrus build rejects Drain instructions with >1 sync wait.
def _patched_drain_and_barrier(self, tick_clock, wait_clock):
    nc = self.nc
    probe = nc.sync.drain()
    wait_clock.add_sem_waits(
        probe.ins, tile_mod.ScopedClock({None: tick_clock.global_clock})
    )
    si = probe.ins.sync_info
    waits = list(si.on_wait) if si is not None else []
    if len(waits) > 1:
        bb = nc.cur_bb.bb
        popped = bb.instructions.pop()
        assert popped is probe.ins
        by_name = {}
        for h in self.sems.allocated().values():
            by_name[h.name] = h
        for w in waits:
            assert w.wait_mode == "sem-ge-imm", w
            nc.sync.wait_ge(by_name[w.ant_name], w.wait_value)
        nc.sync.drain()
    nc.all_engine_barrier()
    popped_p = nc._tile_sem_poison_stack.pop()
    assert popped_p is self._sem_poison
    nc.clear_and_free_semaphores(list(self.sems.allocated().values()))
    nc.all_engine_barrier()


TileContext._drain_and_barrier = _patched_drain_and_barrier

_wsplit_n = 0


def _split_multi_waits(nc, keep=1):
    """This walrus build allows at most one sync-wait per instruction; hoist
    extra waits onto dedicated EventSemaphore instructions just before."""
    global _wsplit_n
    for f in nc.m.functions:
        for bb in f.blocks:
            need = any(
                inst.sync_info is not None and len(inst.sync_info.on_wait) > keep
                for inst in bb.instructions
            )
            if not need:
                continue
            newlist = []
            for inst in bb.instructions:
                si = inst.sync_info
                if si is not None and len(si.on_wait) > keep:
                    waits = list(si.on_wait)
                    for w in waits[:-keep]:
                        ev = mybir.InstEventSemaphore(
                            name=f"WSPLIT-{_wsplit_n}", ins=[], outs=[])
                        _wsplit_n += 1
                        ev.engine = inst.engine
                        ev.sync_info = mybir.SyncInfo(on_wait=[w], on_update=[])
                        newlist.append(ev)
                    inst.sync_info = mybir.SyncInfo(
                        on_wait=waits[-keep:], on_update=list(si.on_update))
                newlist.append(inst)
            try:
                bb.instructions[:] = newlist
            except TypeError:
                while len(bb.instructions):
                    bb.instructions.pop()
                for inst in newlist:
                    bb.instructions.append(inst)


# ---------------------------------------------------------------------------
def _pack_idx(logical):
    """int16 idx list -> [32, n/16] wrapped/replicated layout for dma_gather."""
    n = len(logical)
    assert n % 16 == 0
    a = np.asarray(logical, np.int16).reshape(n // 16, 16).T
    out = np.empty((32, n // 16), np.int16)
    out[:16] = a
    out[16:] = a
    return out


class Cfg:
    def __init__(self, N, E, lim=32768):
        self.N = N
        self.E = E
        self.NC = 8
        self.IN = 256
        self.HID = 64
        self.H1 = 4
        self.OUT = 64
        self.OWN = N // self.NC
        self.OWNP = -(-self.OWN // 128) * 128
        self.ROWS = self.NC * self.OWNP
        self.LIM = lim                      # rows addressable by one window
        self.TBOFF = max(self.ROWS - lim, 0)  # start row of window B
        self.NT = self.OWNP // 128          # dst tiles per core
        self.L1C = 256                      # table-1 row (bf16): z only
        self.L2C = 128                      # table-2 row: z2(64) el2 er2 pad
        self.SENT_A = self.OWN              # sentinel row (block 0 pad row)
        self.SENT_B = (self.NC - 1) * self.OWNP + self.OWN
        assert self.SENT_B < self.ROWS
        assert self.SENT_B - self.TBOFF < lim


def prep(cfg, x, W1, al1, ar1, b1, W2, al2, ar2, b2, src, dst):
    """Host-side graph partitioning / staging.  Returns (in_maps, sched, post)."""
    N, E, NC = cfg.N, cfg.E, cfg.NC
    IN, HID, H1, OUT = cfg.IN, cfg.HID, cfg.H1, cfg.OUT

    x = np.asarray(x, np.float32)
    src = np.asarray(src)
    dst = np.asarray(dst)
    W1 = np.asarray(W1, np.float32)
    W2 = np.asarray(W2, np.float32)
    al1 = np.asarray(al1, np.float32)
    ar1 = np.asarray(ar1, np.float32)
    al2 = np.asarray(al2, np.float32)
    ar2 = np.asarray(ar2, np.float32)
    b1 = np.asarray(b1, np.float32)
    b2 = np.asarray(b2, np.float32)

    # parameter transforms
    AL1 = np.zeros((H1 * HID, H1), np.float32)
    AR1 = np.zeros((H1 * HID, H1), np.float32)
    for h in range(H1):
        AL1[h * HID:(h + 1) * HID, h] = al1[h]
        AR1[h * HID:(h + 1) * HID, h] = ar1[h]
    el1 = x @ (W1 @ AL1)                                    # [N, 4] host-side
    er1 = x @ (W1 @ AR1)                                    # [N, 4] host-side
    AL2 = al2.reshape(OUT, 1)
    AR2 = ar2.reshape(OUT, 1)
    W2aug = np.concatenate([W2, W2 @ AL2, W2 @ AR2], axis=1)  # [256, 66]

    bf = mybir.dt.np(BF16)
    # ownership: round-robin by global degree rank -> per-tile max degrees
    # align across cores (shared program, minimal padding)
    deg_g = np.bincount(dst, minlength=N)
    grank = np.argsort(-deg_g, kind="stable")
    owner = np.empty(N, np.int64)
    local_rank = np.empty(N, np.int64)
    owner[grank] = np.arange(N) % NC
    local_rank[grank] = np.arange(N) // NC
    nodes_by_core = [grank[c::NC] for c in range(NC)]   # local-rank order

    # per-core degree-descending dst order; table rows follow this order so
    # layer-2 tile outputs write back contiguously.  Two passes: the second
    # restores the A-window-count tiebreak (which needs provisional rows).
    eids = [np.nonzero(owner[dst] == c)[0] for c in range(NC)]
    dlocs = [local_rank[dst[eids[c]]] for c in range(NC)]
    degs = [np.bincount(dlocs[c], minlength=cfg.OWN) for c in range(NC)]

    def _rows_from(orders):
        pos = []
        for c in range(NC):
            inv = np.empty(cfg.OWN, np.int64)
            inv[orders[c]] = np.arange(cfg.OWN)
            pos.append(inv)
        row = np.empty(N, np.int64)
        for c in range(NC):
            sel = owner == np.int64(c)
            row[sel] = cfg.OWNP * c + pos[c][local_rank[sel]]
        return row

    orders = [np.argsort(-degs[c], kind="stable") for c in range(NC)]
    row_v1 = _rows_from(orders)
    rows_src_v1 = row_v1[src]
    orders = []
    for c in range(NC):
        mA_all = np.bincount(dlocs[c][rows_src_v1[eids[c]] < cfg.TBOFF],
                             minlength=cfg.OWN)
        orders.append(np.lexsort((mA_all, degs[c]))[::-1])
    row_of_node = _rows_from(orders)
    perms = [
        np.concatenate(
            [orders[c], np.full(cfg.OWNP - cfg.OWN, orders[c][-1], np.int64)]
        )
        for c in range(NC)
    ]

    # x laid out in table-row order (pad rows stay zero)
    xrow = np.zeros((cfg.ROWS, IN), bf)
    xrow[row_of_node] = x.astype(bf)
    xT = np.ascontiguousarray(xrow.T)                   # [IN, ROWS] bf16

    rows_src = row_of_node[src]

    # per-core A/B schedule
    tiles_ab, per_core_rows, per_core_nodes = [], [], []
    for c in range(NC):
        eid = np.nonzero(owner[dst] == c)[0]
        dloc = local_rank[dst[eid]]
        srows = rows_src[eid]
        snode = src[eid]
        perm = perms[c]
        so = np.argsort(dloc, kind="stable")
        srows_sorted = srows[so]
        snode_sorted = snode[so]
        starts = np.searchsorted(dloc[so], np.arange(cfg.OWN + 1))
        byd_r = [srows_sorted[starts[i]:starts[i + 1]] for i in range(cfg.OWN)]
        byd_n = [snode_sorted[starts[i]:starts[i + 1]] for i in range(cfg.OWN)]
        per_core_rows.append(byd_r)
        per_core_nodes.append(byd_n)
        ab = []
        for t in range(cfg.NT):
            dts = perm[t * 128:(t + 1) * 128]
            mA = np.zeros(128, np.int64)
            mB = np.zeros(128, np.int64)
            dg = np.zeros(128, np.int64)
            for i in range(128):
                if t * 128 + i >= cfg.OWN:
                    continue
                rs = byd_r[dts[i]]
                dg[i] = len(rs)
                mA[i] = int((rs < cfg.TBOFF).sum())
                mB[i] = int((rs >= cfg.LIM).sum())
            ab.append((mA, mB, dg))
        tiles_ab.append(ab)

    # common per-tile (alpha, beta) across cores, minimizing slot count with
    # dma_gather call count as tiebreak (the per-core beta(alpha) curves are
    # monotone, so one global alpha scan is exact)
    Ks = []
    for t in range(cfg.NT):
        a_lo = max(max(int(tiles_ab[c][t][0].max()) for c in range(NC)), 1)
        a_hi = max(max(int(tiles_ab[c][t][2].max()) for c in range(NC)), 1)
        best, bkey = None, None
        for alpha in range(a_lo, a_hi + 1):
            beta = 1
            for c in range(NC):
                mA, mB, dg = tiles_ab[c][t]
                beta = max(beta, int(
                    np.maximum(mB, dg - np.minimum(alpha, dg - mB)).max()))
            key = ((alpha + beta) * 108 +
                   (-(-alpha // 8) + -(-beta // 8)) * 130)
            if bkey is None or key < bkey:
                best, bkey = (alpha, beta), key
        Ks.append(best)
    EC = 4 * sum(a + b for a, b in Ks)

    # build per-core idx + per-slot el arrays
    idx_all, elb_all, erb_all = [], [], []
    for c in range(NC):
        perm = perms[c]
        byd_r = per_core_rows[c]
        byd_n = per_core_nodes[c]
        cols = []
        elb = np.empty((128, EC), np.float32)
        eloff = 0
        for t in range(cfg.NT):
            a_t, b_t = Ks[t]
            K = a_t + b_t
            Aidx = np.full((a_t, 128), cfg.SENT_A, np.int64)
            Bidx = np.full((b_t, 128), cfg.SENT_B - cfg.TBOFF, np.int64)
            nod = np.full((K, 128), -1, np.int64)
            dts = perm[t * 128:(t + 1) * 128]
            for i in range(128):
                if t * 128 + i >= cfg.OWN:
                    continue
                rs = byd_r[dts[i]]
                ns = byd_n[dts[i]]
                isA = rs < cfg.TBOFF
                isB = rs >= cfg.LIM
                flexm = ~isA & ~isB
                nA = min(a_t, int(isA.sum()) + int(flexm.sum()))
                take = nA - int(isA.sum())
                fidx = np.nonzero(flexm)[0]
                Asel = np.concatenate([np.nonzero(isA)[0], fidx[:take]])
                Bsel = np.concatenate([np.nonzero(isB)[0], fidx[take:]])
                assert len(Asel) <= a_t and len(Bsel) <= b_t, (t, i)
                Aidx[:len(Asel), i] = rs[Asel]
                Bidx[:len(Bsel), i] = rs[Bsel] - cfg.TBOFF
                nod[:len(Asel), i] = ns[Asel]
                nod[a_t:a_t + len(Bsel), i] = ns[Bsel]
            assert Aidx.max() < cfg.LIM
            blkA = _pack_idx(Aidx.reshape(-1))
            blkB = _pack_idx(Bidx.reshape(-1))
            blk = np.concatenate([blkA, blkB], axis=1)
            pad = (-blk.shape[1]) % 32        # keep 64B alignment per tile
            if pad:
                blk = np.concatenate(
                    [blk, np.zeros((32, pad), np.int16)], axis=1)
            cols.append(blk)
            # el block layout [128, 4, K]: head-major, contiguous K per head
            blk = np.full((128, 4, K), SENT_EL, np.float32)
            valid = nod >= 0                                  # [K, 128]
            vi = np.nonzero(valid)
            blk[vi[1][:, None], np.arange(4)[None, :], vi[0][:, None]] = \
                el1[nod[vi]]
            elb[:, eloff:eloff + 4 * K] = blk.reshape(128, 4 * K)
            eloff += 4 * K
        idx_all.append(np.concatenate(cols, axis=1))
        elb_all.append(elb)
        # er1 for own dsts in (tile, partition) layout [128, NT*4]
        own_nodes = nodes_by_core[c][perms[c]]              # [OWNP]
        erb = np.ascontiguousarray(
            er1[own_nodes].reshape(cfg.NT, 128, 4).transpose(1, 0, 2)
            .reshape(128, cfg.NT * 4)).astype(np.float32)
        erb_all.append(erb)

    b1bc = np.broadcast_to(b1.reshape(1, -1), (128, H1 * HID)).copy()
    b2bc = np.broadcast_to(b2.reshape(1, -1), (128, OUT)).copy()
    ident = np.eye(128, dtype=np.float32)
    pad0 = cfg.OWN - (cfg.NT - 1) * 128
    pcap = np.where(np.arange(128) < pad0, 3.0e38, SENT_EL
                    ).astype(np.float32).reshape(128, 1)

    in_maps = []
    for c in range(NC):
        in_maps.append(
            {
                "xT": xT,
                "W1p": W1.astype(bf),
                "erb": erb_all[c],
                "W2aug": W2aug,
                "b1bc": b1bc,
                "b2bc": b2bc,
                "ident": ident,
                "idx_all": idx_all[c],
                "elb": elb_all[c],
                "pcap": pcap,
            }
        )
    sched = {"Ks": Ks, "idx_cols": idx_all[0].shape[1], "EC": EC}
    post = {"perms": perms, "nodes_by_core": nodes_by_core}
    return in_maps, sched, post


# ---------------------------------------------------------------------------
def build(cfg, sched, debug=False, phases=4, g1_mode=5, reps=1, sp=False,
          preload=True, gbufs=6, g2bufs=8):
    Ks = sched["Ks"]
    EC = sched["EC"]
    nc = bass.Bass()
    IN, H1, HID, OUT = cfg.IN, cfg.H1, cfg.HID, cfg.OUT
    L1C, L2C = cfg.L1C, cfg.L2C
    PAD0 = cfg.OWN - (cfg.NT - 1) * 128     # first pad partition of last tile

    def P(name, shape, dt=F32):
        return nc.declare_dram_parameter(name, list(shape), dt, isOutput=False)

    xT = P("xT", [IN, cfg.ROWS], BF16)
    W1p = P("W1p", [IN, 256], BF16)
    erbp = P("erb", [128, cfg.NT * 4])
    W2a = P("W2aug", [IN, 66])
    b1b = P("b1bc", [128, 256])
    b2b = P("b2bc", [128, OUT])
    idn = P("ident", [128, 128])
    idx_all = P("idx_all", [32, sched["idx_cols"]], I16)
    elbp = P("elb", [128, EC])
    pcapp = P("pcap", [128, 1])
    outp = nc.declare_dram_parameter("outperm", [cfg.OWNP, OUT], F32, isOutput=True)

    tab1 = nc.dram_tensor("tab1", [cfg.ROWS, L1C], BF16)
    shard = nc.dram_tensor("shard", [cfg.OWNP, L2C], BF16)
    tab2 = nc.dram_tensor("tab2", [cfg.ROWS, L2C], BF16, addr_space="Shared")

    _regs = {}

    def nreg(v):
        if v not in _regs:
            _regs[v] = nc.gpsimd.to_reg(v)
        return _regs[v]

    with TileContext(nc) as tc:
        nc.gpsimd.load_library(library_config.mlp)
        with tc.tile_pool(name="const", bufs=1) as cp:
            w1a = cp.tile([128, 2 * 256], BF16, tag="w1a")
            w2a = cp.tile([128, 2 * 66], F32, tag="w2a")
            b1s = cp.tile([128, 256], F32, tag="b1s")
            b2s = cp.tile([128, OUT], F32, tag="b2s")
            ids = cp.tile([128, 128], F32, tag="ids")
            er1 = cp.tile([128, cfg.NT * 4], F32, tag="er1")
            er2 = cp.tile([128, cfg.NT], F32, tag="er2")
            els = cp.tile([128, EC], F32, tag="els")
            pcap = cp.tile([128, 1], F32, tag="pcap")
            itall = cp.tile([32, sched["idx_cols"]], I16, tag="itall")
            for k in range(2):
                nc.sync.dma_start(out=w1a[:, k * 256:(k + 1) * 256],
                                  in_=W1p[k * 128:(k + 1) * 128, :])
                nc.sync.dma_start(out=w2a[:, k * 66:(k + 1) * 66],
                                  in_=W2a[k * 128:(k + 1) * 128, :])
            nc.sync.dma_start(out=b1s[:], in_=b1b[:])
            nc.sync.dma_start(out=b2s[:], in_=b2b[:])
            nc.sync.dma_start(out=ids[:], in_=idn[:])
            nc.sync.dma_start(out=els[:], in_=elbp[:])
            nc.sync.dma_start(out=pcap[:], in_=pcapp[:])
            nc.sync.dma_start(out=itall[:], in_=idx_all[:])
            nc.sync.dma_start(out=er1[:], in_=erbp[:])

            for _rep in range(reps):
                # ---------------- phase Z: z table + er1 ----------------
                SUP = 8  # z tiles per x load
                with tc.tile_pool(name="zx", bufs=3) as zxp, \
                     tc.tile_pool(name="zs", bufs=6) as zsp, \
                     tc.tile_pool(name="zp", bufs=4, space="PSUM") as zpp:
                    NRT = cfg.ROWS // 128
                    for st in range(-(-NRT // SUP)):
                        t0 = st * SUP
                        ntl = min(SUP, NRT - t0)
                        cols = ntl * 128
                        xb = zxp.tile([128, 2, cols], BF16, tag="xb")
                        for k in range(2):
                            nc.sync.dma_start(
                                out=xb[:, k, :],
                                in_=xT[k * 128:(k + 1) * 128,
                                       t0 * 128:t0 * 128 + cols])
                        zw = zsp.tile([128, ntl * L1C], BF16, tag="zw")
                        for i in range(ntl):
                            zp_ = zpp.tile([128, 256], F32, tag="zp")
                            for k in range(2):
                                nc.tensor.matmul(
                                    zp_[:], xb[:, k, i * 128:(i + 1) * 128],
                                    w1a[:, k * 256:(k + 1) * 256],
                                    start=(k == 0), stop=(k == 1))
                            nc.scalar.copy(zw[:, i * L1C:(i + 1) * L1C], zp_[:])
                        r0 = t0 * 128
                        zwa = zw[:]
                        dst_ap = bass.AP(
                            tab1, r0 * L1C,
                            [[L1C, 128], [128 * L1C, ntl], [1, L1C]])
                        src_ap = bass.AP(
                            zwa.tensor, zwa.offset,
                            [zwa.ap[0], [L1C, ntl], [1, L1C]])
                        nc.sync.dma_start(out=dst_ap, in_=src_ap)

                # ---------------- phase G1 + T: layer 1 + z2 ----------------
                Kmax = max(a + b for a, b in Ks)
                ioff = 0
                eloff = 0
                with tc.tile_pool(name="g1", bufs=gbufs) as gp, \
                     tc.tile_pool(name="w1p", bufs=3) as wp, \
                     tc.tile_pool(name="ix", bufs=3) as ip, \
                     tc.tile_pool(name="hb", bufs=2) as hp, \
                     tc.tile_pool(name="s2", bufs=2) as s2p, \
                     tc.tile_pool(name="sm", bufs=4) as smp, \
                     tc.tile_pool(name="tp", bufs=2, space="PSUM") as tpp:
                    for t in range(cfg.NT if phases >= 2 else 0):
                        a_t, b_t = Ks[t]
                        K = a_t + b_t
                        icols = -(-K * 8 // 32) * 32
                        if preload:
                            itv = itall[:, ioff:]
                        else:
                            itt = ip.tile([32, K * 8], I16, tag="it")
                            nc.sync.dma_start(
                                out=itt[:], in_=idx_all[:, ioff:ioff + K * 8])
                            itv = itt[:]
                        g = gp.tile([128, Kmax * L1C], BF16, tag="g")
                        gv = g[:, :K * L1C].rearrange("p (k c) -> p k c", c=L1C)
                        # dma_gather crashes the device above ~1024 idxs/instr;
                        # split into <=8-chunk (1024-idx) pieces.
                        for c0 in range(0, a_t, 8):
                            n = min(8, a_t - c0)
                            nc.gpsimd.dma_gather(
                                out_ap=gv[:, c0:c0 + n, :], in_ap=tab1[:],
                                idxs_ap=itv[:, c0 * 8:(c0 + n) * 8],
                                num_idxs=128 * n,
                                num_idxs_reg=nreg(128 * n), elem_size=L1C,
                                single_packet=sp)
                        for c0 in range(0, b_t, 8):
                            n = min(8, b_t - c0)
                            nc.gpsimd.dma_gather(
                                out_ap=gv[:, a_t + c0:a_t + c0 + n, :],
                                in_ap=tab1[cfg.TBOFF:, :],
                                idxs_ap=itv[:, (a_t + c0) * 8:(a_t + c0 + n) * 8],
                                num_idxs=128 * n,
                                num_idxs_reg=nreg(128 * n), elem_size=L1C,
                                single_packet=sp)
                        # scores
                        if g1_mode < 1:
                            ioff += icols
                            eloff += 4 * K
                            continue
                        w = wp.tile([128, Kmax * 4], BF16, tag="w")
                        wv = w[:, :K * 4].rearrange("p (k h) -> p k h", h=4)
                        s = smp.tile([128, 4], F32, tag="s")
                        rs = smp.tile([128, 4], F32, tag="rs")
                        for h in range(4):
                            nc.scalar.activation(
                                wv[:, :, h], els[:, eloff + h * K:eloff + (h + 1) * K],
                                AF.Prelu,
                                bias=er1[:, 4 * t + h:4 * t + h + 1],
                                scale=1.0, alpha=NEG_SLOPE)
                            nc.scalar.activation(
                                wv[:, :, h], wv[:, :, h], AF.Exp,
                                accum_out=s[:, h:h + 1])
                        nc.vector.tensor_scalar_max(s[:], s[:], 1e-30)
                        nc.vector.reciprocal(rs[:], s[:])
                        # weight messages in place ([K, 4, 64] view over rows)
                        ga = g[:]
                        wa = w[:]
                        gz = bass.AP(ga.tensor, ga.offset,
                                     [ga.ap[0], [L1C, K], [HID, 4], [1, HID]])
                        wbc = bass.AP(wa.tensor, wa.offset,
                                      [wa.ap[0], [4, K], [1, 4], [0, HID]])
                        if g1_mode >= 2:
                            nc.vector.tensor_tensor(gz, gz, wbc, op=ALU.mult)
                        # fold over slots
                        Kc = K
                        while Kc > 1 and g1_mode >= 3:
                            half = Kc // 2
                            m = Kc - half
                            lo = bass.AP(ga.tensor, ga.offset,
                                         [ga.ap[0], [L1C, half], [1, 256]])
                            hi = bass.AP(ga.tensor, ga.offset + m * L1C,
                                         [ga.ap[0], [L1C, half], [1, 256]])
                            nc.vector.tensor_tensor(lo, lo, hi, op=ALU.add)
                            Kc = m
                        # epilogue: h = elu(acc * (1/s) + b1)
                        if g1_mode < 4:
                            ioff += icols
                            eloff += 4 * K
                            continue
                        hb = hp.tile([128, 256], F32, tag="hb")
                        acc = bass.AP(ga.tensor, ga.offset,
                                      [ga.ap[0], [HID, 4], [1, HID]])
                        hba = hb[:]
                        hb4 = bass.AP(hba.tensor, hba.offset,
                                      [hba.ap[0], [HID, 4], [1, HID]])
                        rsa = rs[:]
                        rsb = bass.AP(rsa.tensor, rsa.offset,
                                      [rsa.ap[0], [1, 4], [0, HID]])
                        nc.vector.tensor_tensor(hb4, acc, rsb, op=ALU.mult)
                        nc.vector.tensor_tensor(hb[:], hb[:], b1s[:], op=ALU.add)
                        tmp = hp.tile([128, 256], F32, tag="elutmp")
                        nc.vector.tensor_scalar_min(tmp[:], hb[:], 0.0)
                        nc.vector.tensor_scalar_max(hb[:], hb[:], 0.0)
                        nc.scalar.activation(tmp[:], tmp[:], AF.Exp)
                        nc.vector.tensor_tensor(hb[:], hb[:], tmp[:], op=ALU.add)
                        nc.vector.tensor_scalar_add(hb[:], hb[:], -1.0)
                        # transpose + layer-2 projection
                        if g1_mode < 5:
                            ioff += icols
                            eloff += 4 * K
                            continue
                        zp2 = tpp.tile([128, 66], F32, tag="z2p")
                        for k in range(2):
                            tp = tpp.tile([128, 128], F32, tag="tp")
                            nc.tensor.transpose(tp[:], hb[:, k * 128:(k + 1) * 128],
                                                ids[:])
                            hT = s2p.tile([128, 128], F32, tag="hT")
                            nc.scalar.copy(hT[:], tp[:])
                            nc.tensor.matmul(zp2[:], hT[:],
                                             w2a[:, k * 66:(k + 1) * 66],
                                             start=(k == 0), stop=(k == 1))
                        # tab2 row: [z2 64xbf16 | el2 f32-in-2-bf16-slots | pad]
                        z2sb = s2p.tile([128, L2C], BF16, tag="z2sb")
                        nc.scalar.copy(z2sb[:, 0:64], zp2[:, 0:64])
                        z2f = z2sb[:].bitcast(F32)        # [128, 64] f32 view
                        nc.vector.tensor_copy(z2f[:, 32:33], zp2[:, 64:65])
                        nc.vector.tensor_copy(er2[:, t:t + 1], zp2[:, 65:66])
                        if t == cfg.NT - 1:
                            # pad rows: force el2 so layer-2 pad slots weigh 0
                            nc.vector.tensor_tensor(
                                z2f[:, 32:33], z2f[:, 32:33], pcap[:],
                                op=ALU.min)
                        nc.sync.dma_start(
                            out=shard[t * 128:(t + 1) * 128, :], in_=z2sb[:])
                        ioff += icols
                        eloff += 4 * K

                # ---------------- allgather ----------------
                if phases >= 3:
                    nc.gpsimd.collective_compute(
                        "AllGather", ALU.bypass, ins=[shard[:]], outs=[tab2[:]],
                        replica_groups=[list(range(cfg.NC))])

                # ---------------- phase G2: layer 2 ----------------
                ioff = 0
                with tc.tile_pool(name="g2", bufs=g2bufs) as gp2, \
                     tc.tile_pool(name="w2p", bufs=2) as wp2, \
                     tc.tile_pool(name="ix2", bufs=3) as ip2, \
                     tc.tile_pool(name="ob", bufs=2) as op_, \
                     tc.tile_pool(name="sm2", bufs=4) as smp2:
                    for t in range(cfg.NT if phases >= 4 else 0):
                        a_t, b_t = Ks[t]
                        K = a_t + b_t
                        icols = -(-K * 8 // 32) * 32
                        if preload:
                            itv = itall[:, ioff:]
                        else:
                            itt = ip2.tile([32, K * 8], I16, tag="it2")
                            nc.sync.dma_start(
                                out=itt[:], in_=idx_all[:, ioff:ioff + K * 8])
                            itv = itt[:]
                        g = gp2.tile([128, Kmax * L2C], BF16, tag="g2")
                        gv = g[:, :K * L2C].rearrange("p (k c) -> p k c", c=L2C)
                        for c0 in range(0, a_t, 8):
                            n = min(8, a_t - c0)
                            nc.gpsimd.dma_gather(
                                out_ap=gv[:, c0:c0 + n, :], in_ap=tab2[:],
                                idxs_ap=itv[:, c0 * 8:(c0 + n) * 8],
                                num_idxs=128 * n,
                                num_idxs_reg=nreg(128 * n), elem_size=L2C,
                                single_packet=sp)
                        for c0 in range(0, b_t, 8):
                            n = min(8, b_t - c0)
                            nc.gpsimd.dma_gather(
                                out_ap=gv[:, a_t + c0:a_t + c0 + n, :],
                                in_ap=tab2[cfg.TBOFF:, :],
                                idxs_ap=itv[:, (a_t + c0) * 8:(a_t + c0 + n) * 8],
                                num_idxs=128 * n,
                                num_idxs_reg=nreg(128 * n), elem_size=L2C,
                                single_packet=sp)
                        w2t = wp2.tile([128, Kmax], F32, tag="w2t")
                        s2 = smp2.tile([128, 1], F32, tag="s2")
                        rs2 = smp2.tile([128, 1], F32, tag="rs2")
                        ga = g[:]
                        gf = g[:].bitcast(F32)            # [128, Kmax*64]
                        el2 = bass.AP(gf.tensor, gf.offset + 32,
                                      [gf.ap[0], [L2C // 2, K]])
                        nc.scalar.activation(
                            w2t[:, :K], el2, AF.Prelu,
                            bias=er2[:, t:t + 1], scale=1.0, alpha=NEG_SLOPE)
                        nc.scalar.activation(
                            w2t[:, :K], w2t[:, :K], AF.Exp, accum_out=s2[:])
                        nc.vector.tensor_scalar_max(s2[:], s2[:], 1e-30)
                        nc.vector.reciprocal(rs2[:], s2[:])
                        # weight into f32 accumulator, then fold in f32
                        gw = wp2.tile([128, Kmax * OUT], F32, tag="gw")
                        gwa = gw[:]
                        wa = w2t[:]
                        gz = bass.AP(ga.tensor, ga.offset,
                                     [ga.ap[0], [L2C, K], [1, OUT]])
                        gwz = bass.AP(gwa.tensor, gwa.offset,
                                      [gwa.ap[0], [OUT, K], [1, OUT]])
                        wbc = bass.AP(wa.tensor, wa.offset,
                                      [wa.ap[0], [1, K], [0, OUT]])
                        nc.vector.tensor_tensor(gwz, gz, wbc, op=ALU.mult)
                        Kc = K
                        while Kc > 1:
                            half = Kc // 2
                            m = Kc - half
                            lo = bass.AP(gwa.tensor, gwa.offset,
                                         [gwa.ap[0], [OUT, half], [1, OUT]])
                            hi = bass.AP(gwa.tensor, gwa.offset + m * OUT,
                                         [gwa.ap[0], [OUT, half], [1, OUT]])
                            nc.vector.tensor_tensor(lo, lo, hi, op=ALU.add)
                            Kc = m
                        ob = op_.tile([128, OUT], F32, tag="ob")
                        nc.vector.tensor_scalar_mul(ob[:], gw[:, 0:OUT], rs2[:])
                        nc.vector.tensor_tensor(ob[:], ob[:], b2s[:], op=ALU.add)
                        nc.sync.dma_start(
                            out=outp[t * 128:(t + 1) * 128, :], in_=ob[:])
                        ioff += icols

    _split_multi_waits(nc)
    lower_extended_insts(nc)
    return nc


# ---------------------------------------------------------------------------
_memo = {}


def run(cfg, inputs, trace=False, debug=False):
    in_maps, sched, post = prep(cfg, **inputs)
    key = (cfg.N, cfg.E, cfg.LIM, tuple(sched["Ks"]), bool(debug))
    if key not in _memo:
        _memo[key] = build(cfg, sched, debug=debug)
    nc = _memo[key]
    res = run_bass_kernel_spmd(
        nc, in_maps, list(range(cfg.NC)), trace=trace)
    out = np.zeros((cfg.N, cfg.OUT), np.float32)
    for c in range(cfg.NC):
        op = res.results[c]["outperm"]
        perm = post["perms"][c]
        out[post["nodes_by_core"][c][perm[:cfg.OWN]]] = op[:cfg.OWN]
    return out, res


def run_bench(cfg, inputs, iters=3, reps=1):
    """Run once for outputs, then time repeated executions of the compiled
    NEFF (inputs pre-staged on device, outputs donated fresh each iter).
    reps repeats the whole pipeline inside one NEFF; timing two different
    reps values isolates per-iteration device time from launch latency."""
    import time

    import jax
    from jax.experimental.shard_map import shard_map
    from jax.sharding import Mesh, PartitionSpec

    from concourse import bass2jax

    bass2jax.install_neuronx_cc_hook()

    in_maps, sched, post = prep(cfg, **inputs)
    key = (cfg.N, cfg.E, cfg.LIM, tuple(sched["Ks"]), False, reps)
    if key not in _memo:
        _memo[key] = build(cfg, sched, reps=reps)
    nc = _memo[key]

    partition_name = nc.partition_id_tensor.name if nc.partition_id_tensor else None
    in_names, out_names, out_avals, zero_outs = [], [], [], []
    for alloc in nc.m.functions[0].allocations:
        if not isinstance(alloc, mybir.MemoryLocationSet):
            continue
        name = alloc.memorylocations[0].name
        if alloc.kind == "ExternalInput":
            if name != partition_name:
                in_names.append(name)
        elif alloc.kind == "ExternalOutput":
            out_names.append(name)
            shape = tuple(alloc.tensor_shape)
            dtype = mybir.dt.np(alloc.dtype)
            out_avals.append(jax.core.ShapedArray(shape, dtype))
            zero_outs.append(np.zeros(shape, dtype))
    n_params = len(in_names)
    n_outs = len(out_avals)
    all_in_names = list(in_names) + list(out_names)
    if partition_name is not None:
        all_in_names.append(partition_name)
    donate = tuple(range(n_params, n_params + n_outs))

    def _body(*args):
        operands = list(args)
        if partition_name is not None:
            operands.append(bass2jax.partition_id_tensor())
        outs = bass2jax._bass_exec_p.bind(
            *operands,
            out_avals=tuple(out_avals),
            in_names=tuple(all_in_names),
            out_names=tuple(out_names),
            lowering_input_output_aliases=(),
            sim_require_finite=True,
            sim_require_nnan=True,
            nc=nc,
        )
        return tuple(outs)

    NCOR = cfg.NC
    devices = jax.devices()[:NCOR]
    mesh = Mesh(np.asarray(devices), ("core",))
    in_specs = (PartitionSpec("core"),) * (n_params + n_outs)
    out_specs = (PartitionSpec("core"),) * len(out_names)
    sharded = jax.jit(
        shard_map(_body, mesh=mesh, in_specs=in_specs, out_specs=out_specs,
                  check_rep=False),
        donate_argnums=donate, keep_unused=True)
    sharding = jax.sharding.NamedSharding(mesh, PartitionSpec("core"))
    concat_in = [
        jax.device_put(
            np.concatenate([np.asarray(in_maps[c][n]) for c in range(NCOR)],
                           axis=0), sharding)
        for n in in_names
    ]

    def fresh_zeros():
        return [
            jax.device_put(
                np.zeros((NCOR * z.shape[0], *z.shape[1:]), z.dtype), sharding)
            for z in zero_outs
        ]

    out_arrs = sharded(*concat_in, *fresh_zeros())
    jax.block_until_ready(out_arrs)
    results = [
        {n: np.asarray(out_arrs[i]).reshape(NCOR, *out_avals[i].shape)[c]
         for i, n in enumerate(out_names)}
        for c in range(NCOR)
    ]
    times = []
    for _ in range(iters):
        zs = fresh_zeros()
        jax.block_until_ready(zs)
        t0 = time.perf_counter()
        o = sharded(*concat_in, *zs)
        jax.block_until_ready(o)
        times.append(time.perf_counter() - t0)

    out = np.zeros((cfg.N, cfg.OUT), np.float32)
    for c in range(NCOR):
        op = results[c]["outperm"]
        perm = post["perms"][c]
        out[post["nodes_by_core"][c][perm[:cfg.OWN]]] = op[:cfg.OWN]
    return out, times


def kernel(**inputs):
    cfg = Cfg(N=50000, E=800000)
    out, _ = run(cfg, inputs, trace=False)
    return out


# revision 33
# speedup vs baseline: 1.0350x; 1.0350x over previous
"""2-layer GAT (DGL GATConv-style) on 8 Trainium2 NeuronCores.

Strategy (dst-sharded message passing):
  - Nodes are partitioned across 8 cores round-robin by global in-degree
    rank; core c owns ~6250 dsts and all edges pointing into them.
  - Per-core, own dsts are degree-sorted and tiled 128-at-a-time.  The HBM
    z-table rows are laid out in exactly this (core, tile, slot) order, so
    layer-2 tile results can be written back CONTIGUOUSLY (no scatter), and
    both layers share one set of gather indices.
  - Layer-1 projection z = x @ W1 is computed replicated on every core and
    written to an HBM gather table of 512B bf16 rows.  The per-edge source
    attention term el1[src] is precomputed on the HOST (x @ W1 @ a_l is
    cheap) and staged directly in per-slot layout, so gather rows carry
    only z.
  - Per tile, source rows are fetched with the dma_gather custom DMA
    (int16 indices -> the table is addressed via two overlapping windows
    A=[0,32768) and B=[ROWS-32768,ROWS)).  Edge softmax runs on the
    gathered tile: w = exp(lrelu(el_src + er_dst)), denominators via ACT
    accum, weighting + segment sum on the vector engine (in-place multiply
    + binary-tree folds over the slot axis).
  - Layer-2 projection z2aug = h @ [W2|W2@AL2|W2@AR2] is computed from the
    layer-1 tiles (PE transpose), written contiguously into a per-core
    shard, exchanged with an AllGather collective, then the same
    gather/softmax machinery produces the output.
"""
import sys

sys.path.insert(0, "/opt/trn_rl_repo")

import numpy as np

import concourse.bass as bass
import concourse.mybir as mybir
import concourse.tile as tile_mod
from concourse import library_config
from concourse.library_overlay import lower_extended_insts
from concourse.tile import TileContext
from concourse.bass_utils import run_bass_kernel_spmd

F32 = mybir.dt.float32
BF16 = mybir.dt.bfloat16
I16 = mybir.dt.int16
AF = mybir.ActivationFunctionType
ALU = mybir.AluOpType

NEG_SLOPE = 0.2
SENT_EL = -1.0e30


# ---------------------------------------------------------------------------
# Workaround: this walrus build rejects Drain instructions with >1 sync wait.
def _patched_drain_and_barrier(self, tick_clock, wait_clock):
    nc = self.nc
    probe = nc.sync.drain()
    wait_clock.add_sem_waits(
        probe.ins, tile_mod.ScopedClock({None: tick_clock.global_clock})
    )
    si = probe.ins.sync_info
    waits = list(si.on_wait) if si is not None else []
    if len(waits) > 1:
        bb = nc.cur_bb.bb
        popped = bb.instructions.pop()
        assert popped is probe.ins
        by_name = {}
        for h in self.sems.allocated().values():
            by_name[h.name] = h
        for w in waits:
            assert w.wait_mode == "sem-ge-imm", w
            nc.sync.wait_ge(by_name[w.ant_name], w.wait_value)
        nc.sync.drain()
    nc.all_engine_barrier()
    popped_p = nc._tile_sem_poison_stack.pop()
    assert popped_p is self._sem_poison
    nc.clear_and_free_semaphores(list(self.sems.allocated().values()))
    nc.all_engine_barrier()


TileContext._drain_and_barrier = _patched_drain_and_barrier

_wsplit_n = 0


def _split_multi_waits(nc, keep=1):
    """This walrus build allows at most one sync-wait per instruction; hoist
    extra waits onto dedicated EventSemaphore instructions just before."""
    global _wsplit_n
    for f in nc.m.functions:
        for bb in f.blocks:
            need = any(
                inst.sync_info is not None and len(inst.sync_info.on_wait) > keep
                for inst in bb.instructions
            )
            if not need:
                continue
            newlist = []
            for inst in bb.instructions:
                si = inst.sync_info
                if si is not None and len(si.on_wait) > keep:
                    waits = list(si.on_wait)
                    for w in waits[:-keep]:
                        ev = mybir.InstEventSemaphore(
                            name=f"WSPLIT-{_wsplit_n}", ins=[], outs=[])
                        _wsplit_n += 1
                        ev.engine = inst.engine
                        ev.sync_info = mybir.SyncInfo(on_wait=[w], on_update=[])
                        newlist.append(ev)
                    inst.sync_info = mybir.SyncInfo(
                        on_wait=waits[-keep:], on_update=list(si.on_update))
                newlist.append(inst)
            try:
                bb.instructions[:] = newlist
            except TypeError:
                while len(bb.instructions):
                    bb.instructions.pop()
                for inst in newlist:
                    bb.instructions.append(inst)


# ---------------------------------------------------------------------------
def _pack_idx(logical):
    """int16 idx list -> [32, n/16] wrapped/replicated layout for dma_gather."""
    n = len(logical)
    assert n % 16 == 0
    a = np.asarray(logical, np.int16).reshape(n // 16, 16).T
    out = np.empty((32, n // 16), np.int16)
    out[:16] = a
    out[16:] = a
    return out


class Cfg:
    def __init__(self, N, E, lim=32768):
        self.N = N
        self.E = E
        self.NC = 8
        self.IN = 256
        self.HID = 64
        self.H1 = 4
        self.OUT = 64
        self.OWN = N // self.NC
        self.OWNP = -(-self.OWN // 128) * 128
        self.ROWS = self.NC * self.OWNP
        self.LIM = lim                      # rows addressable by one window
        self.TBOFF = max(self.ROWS - lim, 0)  # start row of window B
        self.NT = self.OWNP // 128          # dst tiles per core
        self.L1C = 256                      # table-1 row (bf16): z only
        self.L2C = 128                      # table-2 row: z2(64) el2 er2 pad
        self.SENT_A = self.OWN              # sentinel row (block 0 pad row)
        self.SENT_B = (self.NC - 1) * self.OWNP + self.OWN
        assert self.SENT_B < self.ROWS
        assert self.SENT_B - self.TBOFF < lim


def prep(cfg, x, W1, al1, ar1, b1, W2, al2, ar2, b2, src, dst):
    """Host-side graph partitioning / staging.  Returns (in_maps, sched, post)."""
    N, E, NC = cfg.N, cfg.E, cfg.NC
    IN, HID, H1, OUT = cfg.IN, cfg.HID, cfg.H1, cfg.OUT

    x = np.asarray(x, np.float32)
    src = np.asarray(src)
    dst = np.asarray(dst)
    W1 = np.asarray(W1, np.float32)
    W2 = np.asarray(W2, np.float32)
    al1 = np.asarray(al1, np.float32)
    ar1 = np.asarray(ar1, np.float32)
    al2 = np.asarray(al2, np.float32)
    ar2 = np.asarray(ar2, np.float32)
    b1 = np.asarray(b1, np.float32)
    b2 = np.asarray(b2, np.float32)

    # parameter transforms
    AL1 = np.zeros((H1 * HID, H1), np.float32)
    AR1 = np.zeros((H1 * HID, H1), np.float32)
    for h in range(H1):
        AL1[h * HID:(h + 1) * HID, h] = al1[h]
        AR1[h * HID:(h + 1) * HID, h] = ar1[h]
    el1 = x @ (W1 @ AL1)                                    # [N, 4] host-side
    er1 = x @ (W1 @ AR1)                                    # [N, 4] host-side
    AL2 = al2.reshape(OUT, 1)
    AR2 = ar2.reshape(OUT, 1)
    W2aug = np.concatenate([W2, W2 @ AL2, W2 @ AR2], axis=1)  # [256, 66]

    bf = mybir.dt.np(BF16)
    # ownership: round-robin by global degree rank -> per-tile max degrees
    # align across cores (shared program, minimal padding)
    deg_g = np.bincount(dst, minlength=N)
    grank = np.argsort(-deg_g, kind="stable")
    owner = np.empty(N, np.int64)
    local_rank = np.empty(N, np.int64)
    owner[grank] = np.arange(N) % NC
    local_rank[grank] = np.arange(N) // NC
    nodes_by_core = [grank[c::NC] for c in range(NC)]   # local-rank order

    # per-core degree-descending dst order; table rows follow this order so
    # layer-2 tile outputs write back contiguously.  Two passes: the second
    # restores the A-window-count tiebreak (which needs provisional rows).
    eids = [np.nonzero(owner[dst] == c)[0] for c in range(NC)]
    dlocs = [local_rank[dst[eids[c]]] for c in range(NC)]
    degs = [np.bincount(dlocs[c], minlength=cfg.OWN) for c in range(NC)]

    def _rows_from(orders):
        pos = []
        for c in range(NC):
            inv = np.empty(cfg.OWN, np.int64)
            inv[orders[c]] = np.arange(cfg.OWN)
            pos.append(inv)
        row = np.empty(N, np.int64)
        for c in range(NC):
            sel = owner == np.int64(c)
            row[sel] = cfg.OWNP * c + pos[c][local_rank[sel]]
        return row

    orders = [np.argsort(-degs[c], kind="stable") for c in range(NC)]
    row_v1 = _rows_from(orders)
    rows_src_v1 = row_v1[src]
    orders = []
    for c in range(NC):
        mA_all = np.bincount(dlocs[c][rows_src_v1[eids[c]] < cfg.TBOFF],
                             minlength=cfg.OWN)
        orders.append(np.lexsort((mA_all, degs[c]))[::-1])
    row_of_node = _rows_from(orders)
    perms = [
        np.concatenate(
            [orders[c], np.full(cfg.OWNP - cfg.OWN, orders[c][-1], np.int64)]
        )
        for c in range(NC)
    ]

    # x laid out in table-row order (pad rows stay zero)
    xrow = np.zeros((cfg.ROWS, IN), bf)
    xrow[row_of_node] = x.astype(bf)
    xT = np.ascontiguousarray(xrow.T)                   # [IN, ROWS] bf16

    rows_src = row_of_node[src]

    # per-core A/B schedule
    tiles_ab, per_core_rows, per_core_nodes = [], [], []
    for c in range(NC):
        eid = np.nonzero(owner[dst] == c)[0]
        dloc = local_rank[dst[eid]]
        srows = rows_src[eid]
        snode = src[eid]
        perm = perms[c]
        so = np.argsort(dloc, kind="stable")
        srows_sorted = srows[so]
        snode_sorted = snode[so]
        starts = np.searchsorted(dloc[so], np.arange(cfg.OWN + 1))
        byd_r = [srows_sorted[starts[i]:starts[i + 1]] for i in range(cfg.OWN)]
        byd_n = [snode_sorted[starts[i]:starts[i + 1]] for i in range(cfg.OWN)]
        per_core_rows.append(byd_r)
        per_core_nodes.append(byd_n)
        ab = []
        for t in range(cfg.NT):
            dts = perm[t * 128:(t + 1) * 128]
            mA = np.zeros(128, np.int64)
            mB = np.zeros(128, np.int64)
            dg = np.zeros(128, np.int64)
            for i in range(128):
                if t * 128 + i >= cfg.OWN:
                    continue
                rs = byd_r[dts[i]]
                dg[i] = len(rs)
                mA[i] = int((rs < cfg.TBOFF).sum())
                mB[i] = int((rs >= cfg.LIM).sum())
            ab.append((mA, mB, dg))
        tiles_ab.append(ab)

    # common per-tile (alpha, beta) across cores, minimizing slot count with
    # dma_gather call count as tiebreak (the per-core beta(alpha) curves are
    # monotone, so one global alpha scan is exact)
    Ks = []
    for t in range(cfg.NT):
        a_lo = max(max(int(tiles_ab[c][t][0].max()) for c in range(NC)), 1)
        a_hi = max(max(int(tiles_ab[c][t][2].max()) for c in range(NC)), 1)
        best, bkey = None, None
        for alpha in range(a_lo, a_hi + 1):
            beta = 1
            for c in range(NC):
                mA, mB, dg = tiles_ab[c][t]
                beta = max(beta, int(
                    np.maximum(mB, dg - np.minimum(alpha, dg - mB)).max()))
            key = ((alpha + beta) * 108 +
                   (-(-alpha // 8) + -(-beta // 8)) * 130)
            if bkey is None or key < bkey:
                best, bkey = (alpha, beta), key
        Ks.append(best)
    EC = 4 * sum(a + b for a, b in Ks)

    # build per-core idx + per-slot el arrays
    idx_all, elb_all, erb_all = [], [], []
    for c in range(NC):
        perm = perms[c]
        byd_r = per_core_rows[c]
        byd_n = per_core_nodes[c]
        cols = []
        elb = np.empty((128, EC), np.float32)
        eloff = 0
        for t in range(cfg.NT):
            a_t, b_t = Ks[t]
            K = a_t + b_t
            Aidx = np.full((a_t, 128), cfg.SENT_A, np.int64)
            Bidx = np.full((b_t, 128), cfg.SENT_B - cfg.TBOFF, np.int64)
            nod = np.full((K, 128), -1, np.int64)
            dts = perm[t * 128:(t + 1) * 128]
            for i in range(128):
                if t * 128 + i >= cfg.OWN:
                    continue
                rs = byd_r[dts[i]]
                ns = byd_n[dts[i]]
                isA = rs < cfg.TBOFF
                isB = rs >= cfg.LIM
                flexm = ~isA & ~isB
                nA = min(a_t, int(isA.sum()) + int(flexm.sum()))
                take = nA - int(isA.sum())
                fidx = np.nonzero(flexm)[0]
                Asel = np.concatenate([np.nonzero(isA)[0], fidx[:take]])
                Bsel = np.concatenate([np.nonzero(isB)[0], fidx[take:]])
                assert len(Asel) <= a_t and len(Bsel) <= b_t, (t, i)
                Aidx[:len(Asel), i] = rs[Asel]
                Bidx[:len(Bsel), i] = rs[Bsel] - cfg.TBOFF
                nod[:len(Asel), i] = ns[Asel]
                nod[a_t:a_t + len(Bsel), i] = ns[Bsel]
            assert Aidx.max() < cfg.LIM
            blkA = _pack_idx(Aidx.reshape(-1))
            blkB = _pack_idx(Bidx.reshape(-1))
            blk = np.concatenate([blkA, blkB], axis=1)
            pad = (-blk.shape[1]) % 32        # keep 64B alignment per tile
            if pad:
                blk = np.concatenate(
                    [blk, np.zeros((32, pad), np.int16)], axis=1)
            cols.append(blk)
            # el block layout [128, 4, K]: head-major, contiguous K per head
            blk = np.full((128, 4, K), SENT_EL, np.float32)
            valid = nod >= 0                                  # [K, 128]
            vi = np.nonzero(valid)
            blk[vi[1][:, None], np.arange(4)[None, :], vi[0][:, None]] = \
                el1[nod[vi]]
            elb[:, eloff:eloff + 4 * K] = blk.reshape(128, 4 * K)
            eloff += 4 * K
        idx_all.append(np.concatenate(cols, axis=1))
        elb_all.append(elb)
        # er1 for own dsts in (tile, partition) layout [128, NT*4]
        own_nodes = nodes_by_core[c][perms[c]]              # [OWNP]
        erb = np.ascontiguousarray(
            er1[own_nodes].reshape(cfg.NT, 128, 4).transpose(1, 0, 2)
            .reshape(128, cfg.NT * 4)).astype(np.float32)
        erb_all.append(erb)

    b1bc = np.broadcast_to(b1.reshape(1, -1), (128, H1 * HID)).copy()
    b2bc = np.broadcast_to(b2.reshape(1, -1), (128, OUT)).copy()
    ident = np.eye(128, dtype=np.float32)
    pad0 = cfg.OWN - (cfg.NT - 1) * 128
    pcap = np.where(np.arange(128) < pad0, 3.0e38, SENT_EL
                    ).astype(np.float32).reshape(128, 1)

    in_maps = []
    for c in range(NC):
        in_maps.append(
            {
                "xT": xT,
                "W1p": W1.astype(bf),
                "erb": erb_all[c],
                "W2aug": W2aug,
                "b1bc": b1bc,
                "b2bc": b2bc,
                "ident": ident,
                "idx_all": idx_all[c],
                "elb": elb_all[c],
                "pcap": pcap,
            }
        )
    sched = {"Ks": Ks, "idx_cols": idx_all[0].shape[1], "EC": EC}
    post = {"perms": perms, "nodes_by_core": nodes_by_core}
    return in_maps, sched, post


# ---------------------------------------------------------------------------
def build(cfg, sched, debug=False, phases=4, g1_mode=5, reps=1, sp=False,
          preload=True, gbufs=6, g2bufs=6):
    Ks = sched["Ks"]
    EC = sched["EC"]
    nc = bass.Bass()
    IN, H1, HID, OUT = cfg.IN, cfg.H1, cfg.HID, cfg.OUT
    L1C, L2C = cfg.L1C, cfg.L2C
    PAD0 = cfg.OWN - (cfg.NT - 1) * 128     # first pad partition of last tile

    def P(name, shape, dt=F32):
        return nc.declare_dram_parameter(name, list(shape), dt, isOutput=False)

    xT = P("xT", [IN, cfg.ROWS], BF16)
    W1p = P("W1p", [IN, 256], BF16)
    erbp = P("erb", [128, cfg.NT * 4])
    W2a = P("W2aug", [IN, 66])
    b1b = P("b1bc", [128, 256])
    b2b = P("b2bc", [128, OUT])
    idn = P("ident", [128, 128])
    idx_all = P("idx_all", [32, sched["idx_cols"]], I16)
    elbp = P("elb", [128, EC])
    pcapp = P("pcap", [128, 1])
    outp = nc.declare_dram_parameter("outperm", [cfg.OWNP, OUT], F32, isOutput=True)

    tab1 = nc.dram_tensor("tab1", [cfg.ROWS, L1C], BF16)
    shard = nc.dram_tensor("shard", [cfg.OWNP, L2C], BF16)
    tab2 = nc.dram_tensor("tab2", [cfg.ROWS, L2C], BF16, addr_space="Shared")

    _regs = {}

    def nreg(v):
        if v not in _regs:
            _regs[v] = nc.gpsimd.to_reg(v)
        return _regs[v]

    with TileContext(nc) as tc:
        nc.gpsimd.load_library(library_config.mlp)
        with tc.tile_pool(name="const", bufs=1) as cp:
            w1a = cp.tile([128, 2 * 256], BF16, tag="w1a")
            w2a = cp.tile([128, 2 * 66], F32, tag="w2a")
            b1s = cp.tile([128, 256], F32, tag="b1s")
            b2s = cp.tile([128, OUT], F32, tag="b2s")
            ids = cp.tile([128, 128], F32, tag="ids")
            er1 = cp.tile([128, cfg.NT * 4], F32, tag="er1")
            er2 = cp.tile([128, cfg.NT], F32, tag="er2")
            els = cp.tile([128, EC], F32, tag="els")
            pcap = cp.tile([128, 1], F32, tag="pcap")
            itall = cp.tile([32, sched["idx_cols"]], I16, tag="itall")
            for k in range(2):
                nc.sync.dma_start(out=w1a[:, k * 256:(k + 1) * 256],
                                  in_=W1p[k * 128:(k + 1) * 128, :])
                nc.sync.dma_start(out=w2a[:, k * 66:(k + 1) * 66],
                                  in_=W2a[k * 128:(k + 1) * 128, :])
            nc.sync.dma_start(out=b1s[:], in_=b1b[:])
            nc.sync.dma_start(out=b2s[:], in_=b2b[:])
            nc.sync.dma_start(out=ids[:], in_=idn[:])
            nc.sync.dma_start(out=els[:], in_=elbp[:])
            nc.sync.dma_start(out=pcap[:], in_=pcapp[:])
            nc.sync.dma_start(out=itall[:], in_=idx_all[:])
            nc.sync.dma_start(out=er1[:], in_=erbp[:])

            for _rep in range(reps):
                # ---------------- phase Z: z table + er1 ----------------
                SUP = 8  # z tiles per x load
                with tc.tile_pool(name="zx", bufs=3) as zxp, \
                     tc.tile_pool(name="zs", bufs=6) as zsp, \
                     tc.tile_pool(name="zp", bufs=4, space="PSUM") as zpp:
                    NRT = cfg.ROWS // 128
                    for st in range(-(-NRT // SUP)):
                        t0 = st * SUP
                        ntl = min(SUP, NRT - t0)
                        cols = ntl * 128
                        xb = zxp.tile([128, 2, cols], BF16, tag="xb")
                        for k in range(2):
                            nc.sync.dma_start(
                                out=xb[:, k, :],
                                in_=xT[k * 128:(k + 1) * 128,
                                       t0 * 128:t0 * 128 + cols])
                        zw = zsp.tile([128, ntl * L1C], BF16, tag="zw")
                        for i in range(ntl):
                            zp_ = zpp.tile([128, 256], F32, tag="zp")
                            for k in range(2):
                                nc.tensor.matmul(
                                    zp_[:], xb[:, k, i * 128:(i + 1) * 128],
                                    w1a[:, k * 256:(k + 1) * 256],
                                    start=(k == 0), stop=(k == 1))
                            nc.scalar.copy(zw[:, i * L1C:(i + 1) * L1C], zp_[:])
                        r0 = t0 * 128
                        zwa = zw[:]
                        dst_ap = bass.AP(
                            tab1, r0 * L1C,
                            [[L1C, 128], [128 * L1C, ntl], [1, L1C]])
                        src_ap = bass.AP(
                            zwa.tensor, zwa.offset,
                            [zwa.ap[0], [L1C, ntl], [1, L1C]])
                        nc.sync.dma_start(out=dst_ap, in_=src_ap)

                # ---------------- phase G1 + T: layer 1 + z2 ----------------
                Kmax = max(a + b for a, b in Ks)
                ioff = 0
                eloff = 0
                with tc.tile_pool(name="g1", bufs=gbufs) as gp, \
                     tc.tile_pool(name="w1p", bufs=3) as wp, \
                     tc.tile_pool(name="ix", bufs=3) as ip, \
                     tc.tile_pool(name="hb", bufs=2) as hp, \
                     tc.tile_pool(name="s2", bufs=2) as s2p, \
                     tc.tile_pool(name="sm", bufs=4) as smp, \
                     tc.tile_pool(name="tp", bufs=2, space="PSUM") as tpp:
                    for t in range(cfg.NT if phases >= 2 else 0):
                        a_t, b_t = Ks[t]
                        K = a_t + b_t
                        icols = -(-K * 8 // 32) * 32
                        if preload:
                            itv = itall[:, ioff:]
                        else:
                            itt = ip.tile([32, K * 8], I16, tag="it")
                            nc.sync.dma_start(
                                out=itt[:], in_=idx_all[:, ioff:ioff + K * 8])
                            itv = itt[:]
                        g = gp.tile([128, Kmax * L1C], BF16, tag="g")
                        gv = g[:, :K * L1C].rearrange("p (k c) -> p k c", c=L1C)
                        # dma_gather crashes the device above ~1024 idxs/instr;
                        # split into <=8-chunk (1024-idx) pieces.
                        for c0 in range(0, a_t, 8):
                            n = min(8, a_t - c0)
                            nc.gpsimd.dma_gather(
                                out_ap=gv[:, c0:c0 + n, :], in_ap=tab1[:],
                                idxs_ap=itv[:, c0 * 8:(c0 + n) * 8],
                                num_idxs=128 * n,
                                num_idxs_reg=nreg(128 * n), elem_size=L1C,
                                single_packet=sp)
                        for c0 in range(0, b_t, 8):
                            n = min(8, b_t - c0)
                            nc.gpsimd.dma_gather(
                                out_ap=gv[:, a_t + c0:a_t + c0 + n, :],
                                in_ap=tab1[cfg.TBOFF:, :],
                                idxs_ap=itv[:, (a_t + c0) * 8:(a_t + c0 + n) * 8],
                                num_idxs=128 * n,
                                num_idxs_reg=nreg(128 * n), elem_size=L1C,
                                single_packet=sp)
                        # scores
                        if g1_mode < 1:
                            ioff += icols
                            eloff += 4 * K
                            continue
                        w = wp.tile([128, Kmax * 4], BF16, tag="w")
                        wv = w[:, :K * 4].rearrange("p (k h) -> p k h", h=4)
                        s = smp.tile([128, 4], F32, tag="s")
                        rs = smp.tile([128, 4], F32, tag="rs")
                        for h in range(4):
                            nc.scalar.activation(
                                wv[:, :, h], els[:, eloff + h * K:eloff + (h + 1) * K],
                                AF.Prelu,
                                bias=er1[:, 4 * t + h:4 * t + h + 1],
                                scale=1.0, alpha=NEG_SLOPE)
                            nc.scalar.activation(
                                wv[:, :, h], wv[:, :, h], AF.Exp,
                                accum_out=s[:, h:h + 1])
                        nc.vector.tensor_scalar_max(s[:], s[:], 1e-30)
                        nc.vector.reciprocal(rs[:], s[:])
                        # weight messages in place ([K, 4, 64] view over rows)
                        ga = g[:]
                        wa = w[:]
                        gz = bass.AP(ga.tensor, ga.offset,
                                     [ga.ap[0], [L1C, K], [HID, 4], [1, HID]])
                        wbc = bass.AP(wa.tensor, wa.offset,
                                      [wa.ap[0], [4, K], [1, 4], [0, HID]])
                        if g1_mode >= 2:
                            nc.vector.tensor_tensor(gz, gz, wbc, op=ALU.mult)
                        # fold over slots
                        Kc = K
                        while Kc > 1 and g1_mode >= 3:
                            half = Kc // 2
                            m = Kc - half
                            lo = bass.AP(ga.tensor, ga.offset,
                                         [ga.ap[0], [L1C, half], [1, 256]])
                            hi = bass.AP(ga.tensor, ga.offset + m * L1C,
                                         [ga.ap[0], [L1C, half], [1, 256]])
                            nc.vector.tensor_tensor(lo, lo, hi, op=ALU.add)
                            Kc = m
                        # epilogue: h = elu(acc * (1/s) + b1)
                        if g1_mode < 4:
                            ioff += icols
                            eloff += 4 * K
                            continue
                        hb = hp.tile([128, 256], F32, tag="hb")
                        acc = bass.AP(ga.tensor, ga.offset,
                                      [ga.ap[0], [HID, 4], [1, HID]])
                        hba = hb[:]
                        hb4 = bass.AP(hba.tensor, hba.offset,
                                      [hba.ap[0], [HID, 4], [1, HID]])
                        rsa = rs[:]
                        rsb = bass.AP(rsa.tensor, rsa.offset,
                                      [rsa.ap[0], [1, 4], [0, HID]])
                        nc.vector.tensor_tensor(hb4, acc, rsb, op=ALU.mult)
                        nc.vector.tensor_tensor(hb[:], hb[:], b1s[:], op=ALU.add)
                        tmp = hp.tile([128, 256], F32, tag="elutmp")
                        nc.vector.tensor_scalar_min(tmp[:], hb[:], 0.0)
                        nc.vector.tensor_scalar_max(hb[:], hb[:], 0.0)
                        nc.scalar.activation(tmp[:], tmp[:], AF.Exp)
                        nc.vector.tensor_tensor(hb[:], hb[:], tmp[:], op=ALU.add)
                        nc.vector.tensor_scalar_add(hb[:], hb[:], -1.0)
                        # transpose + layer-2 projection
                        if g1_mode < 5:
                            ioff += icols
                            eloff += 4 * K
                            continue
                        zp2 = tpp.tile([128, 66], F32, tag="z2p")
                        for k in range(2):
                            tp = tpp.tile([128, 128], F32, tag="tp")
                            nc.tensor.transpose(tp[:], hb[:, k * 128:(k + 1) * 128],
                                                ids[:])
                            hT = s2p.tile([128, 128], F32, tag="hT")
                            nc.scalar.copy(hT[:], tp[:])
                            nc.tensor.matmul(zp2[:], hT[:],
                                             w2a[:, k * 66:(k + 1) * 66],
                                             start=(k == 0), stop=(k == 1))
                        # tab2 row: [z2 64xbf16 | el2 f32-in-2-bf16-slots | pad]
                        z2sb = s2p.tile([128, L2C], BF16, tag="z2sb")
                        nc.scalar.copy(z2sb[:, 0:64], zp2[:, 0:64])
                        z2f = z2sb[:].bitcast(F32)        # [128, 64] f32 view
                        nc.vector.tensor_copy(z2f[:, 32:33], zp2[:, 64:65])
                        nc.vector.tensor_copy(er2[:, t:t + 1], zp2[:, 65:66])
                        if t == cfg.NT - 1:
                            # pad rows: force el2 so layer-2 pad slots weigh 0
                            nc.vector.tensor_tensor(
                                z2f[:, 32:33], z2f[:, 32:33], pcap[:],
                                op=ALU.min)
                        nc.sync.dma_start(
                            out=shard[t * 128:(t + 1) * 128, :], in_=z2sb[:])
                        ioff += icols
                        eloff += 4 * K

                # ---------------- allgather ----------------
                if phases >= 3:
                    nc.gpsimd.collective_compute(
                        "AllGather", ALU.bypass, ins=[shard[:]], outs=[tab2[:]],
                        replica_groups=[list(range(cfg.NC))])

                # ---------------- phase G2: layer 2 ----------------
                ioff = 0
                with tc.tile_pool(name="g2", bufs=g2bufs) as gp2, \
                     tc.tile_pool(name="w2p", bufs=2) as wp2, \
                     tc.tile_pool(name="ix2", bufs=3) as ip2, \
                     tc.tile_pool(name="ob", bufs=2) as op_, \
                     tc.tile_pool(name="sm2", bufs=4) as smp2:
                    for t in range(cfg.NT if phases >= 4 else 0):
                        a_t, b_t = Ks[t]
                        K = a_t + b_t
                        icols = -(-K * 8 // 32) * 32
                        if preload:
                            itv = itall[:, ioff:]
                        else:
                            itt = ip2.tile([32, K * 8], I16, tag="it2")
                            nc.sync.dma_start(
                                out=itt[:], in_=idx_all[:, ioff:ioff + K * 8])
                            itv = itt[:]
                        g = gp2.tile([128, Kmax * L2C], BF16, tag="g2")
                        gv = g[:, :K * L2C].rearrange("p (k c) -> p k c", c=L2C)
                        for c0 in range(0, a_t, 8):
                            n = min(8, a_t - c0)
                            nc.gpsimd.dma_gather(
                                out_ap=gv[:, c0:c0 + n, :], in_ap=tab2[:],
                                idxs_ap=itv[:, c0 * 8:(c0 + n) * 8],
                                num_idxs=128 * n,
                                num_idxs_reg=nreg(128 * n), elem_size=L2C,
                                single_packet=sp)
                        for c0 in range(0, b_t, 8):
                            n = min(8, b_t - c0)
                            nc.gpsimd.dma_gather(
                                out_ap=gv[:, a_t + c0:a_t + c0 + n, :],
                                in_ap=tab2[cfg.TBOFF:, :],
                                idxs_ap=itv[:, (a_t + c0) * 8:(a_t + c0 + n) * 8],
                                num_idxs=128 * n,
                                num_idxs_reg=nreg(128 * n), elem_size=L2C,
                                single_packet=sp)
                        w2t = wp2.tile([128, Kmax], F32, tag="w2t")
                        s2 = smp2.tile([128, 1], F32, tag="s2")
                        rs2 = smp2.tile([128, 1], F32, tag="rs2")
                        ga = g[:]
                        gf = g[:].bitcast(F32)            # [128, Kmax*64]
                        el2 = bass.AP(gf.tensor, gf.offset + 32,
                                      [gf.ap[0], [L2C // 2, K]])
                        nc.scalar.activation(
                            w2t[:, :K], el2, AF.Prelu,
                            bias=er2[:, t:t + 1], scale=1.0, alpha=NEG_SLOPE)
                        nc.scalar.activation(
                            w2t[:, :K], w2t[:, :K], AF.Exp, accum_out=s2[:])
                        nc.vector.tensor_scalar_max(s2[:], s2[:], 1e-30)
                        nc.vector.reciprocal(rs2[:], s2[:])
                        # weight into f32 accumulator, then fold in f32
                        gw = wp2.tile([128, Kmax * OUT], F32, tag="gw")
                        gwa = gw[:]
                        wa = w2t[:]
                        gz = bass.AP(ga.tensor, ga.offset,
                                     [ga.ap[0], [L2C, K], [1, OUT]])
                        gwz = bass.AP(gwa.tensor, gwa.offset,
                                      [gwa.ap[0], [OUT, K], [1, OUT]])
                        wbc = bass.AP(wa.tensor, wa.offset,
                                      [wa.ap[0], [1, K], [0, OUT]])
                        nc.vector.tensor_tensor(gwz, gz, wbc, op=ALU.mult)
                        Kc = K
                        while Kc > 1:
                            half = Kc // 2
                            m = Kc - half
                            lo = bass.AP(gwa.tensor, gwa.offset,
                                         [gwa.ap[0], [OUT, half], [1, OUT]])
                            hi = bass.AP(gwa.tensor, gwa.offset + m * OUT,
                                         [gwa.ap[0], [OUT, half], [1, OUT]])
                            nc.vector.tensor_tensor(lo, lo, hi, op=ALU.add)
                            Kc = m
                        ob = op_.tile([128, OUT], F32, tag="ob")
                        nc.vector.tensor_scalar_mul(ob[:], gw[:, 0:OUT], rs2[:])
                        nc.vector.tensor_tensor(ob[:], ob[:], b2s[:], op=ALU.add)
                        nc.sync.dma_start(
                            out=outp[t * 128:(t + 1) * 128, :], in_=ob[:])
                        ioff += icols

    _split_multi_waits(nc)
    lower_extended_insts(nc)
    return nc


# ---------------------------------------------------------------------------
_memo = {}


def run(cfg, inputs, trace=False, debug=False):
    in_maps, sched, post = prep(cfg, **inputs)
    key = (cfg.N, cfg.E, cfg.LIM, tuple(sched["Ks"]), bool(debug))
    if key not in _memo:
        _memo[key] = build(cfg, sched, debug=debug)
    nc = _memo[key]
    res = run_bass_kernel_spmd(
        nc, in_maps, list(range(cfg.NC)), trace=trace)
    out = np.zeros((cfg.N, cfg.OUT), np.float32)
    for c in range(cfg.NC):
        op = res.results[c]["outperm"]
        perm = post["perms"][c]
        out[post["nodes_by_core"][c][perm[:cfg.OWN]]] = op[:cfg.OWN]
    return out, res


def run_bench(cfg, inputs, iters=3, reps=1):
    """Run once for outputs, then time repeated executions of the compiled
    NEFF (inputs pre-staged on device, outputs donated fresh each iter).
    reps repeats the whole pipeline inside one NEFF; timing two different
    reps values isolates per-iteration device time from launch latency."""
    import time

    import jax
    from jax.experimental.shard_map import shard_map
    from jax.sharding import Mesh, PartitionSpec

    from concourse import bass2jax

    bass2jax.install_neuronx_cc_hook()

    in_maps, sched, post = prep(cfg, **inputs)
    key = (cfg.N, cfg.E, cfg.LIM, tuple(sched["Ks"]), False, reps)
    if key not in _memo:
        _memo[key] = build(cfg, sched, reps=reps)
    nc = _memo[key]

    partition_name = nc.partition_id_tensor.name if nc.partition_id_tensor else None
    in_names, out_names, out_avals, zero_outs = [], [], [], []
    for alloc in nc.m.functions[0].allocations:
        if not isinstance(alloc, mybir.MemoryLocationSet):
            continue
        name = alloc.memorylocations[0].name
        if alloc.kind == "ExternalInput":
            if name != partition_name:
                in_names.append(name)
        elif alloc.kind == "ExternalOutput":
            out_names.append(name)
            shape = tuple(alloc.tensor_shape)
            dtype = mybir.dt.np(alloc.dtype)
            out_avals.append(jax.core.ShapedArray(shape, dtype))
            zero_outs.append(np.zeros(shape, dtype))
    n_params = len(in_names)
    n_outs = len(out_avals)
    all_in_names = list(in_names) + list(out_names)
    if partition_name is not None:
        all_in_names.append(partition_name)
    donate = tuple(range(n_params, n_params + n_outs))

    def _body(*args):
        operands = list(args)
        if partition_name is not None:
            operands.append(bass2jax.partition_id_tensor())
        outs = bass2jax._bass_exec_p.bind(
            *operands,
            out_avals=tuple(out_avals),
            in_names=tuple(all_in_names),
            out_names=tuple(out_names),
            lowering_input_output_aliases=(),
            sim_require_finite=True,
            sim_require_nnan=True,
            nc=nc,
        )
        return tuple(outs)

    NCOR = cfg.NC
    devices = jax.devices()[:NCOR]
    mesh = Mesh(np.asarray(devices), ("core",))
    in_specs = (PartitionSpec("core"),) * (n_params + n_outs)
    out_specs = (PartitionSpec("core"),) * len(out_names)
    sharded = jax.jit(
        shard_map(_body, mesh=mesh, in_specs=in_specs, out_specs=out_specs,
                  check_rep=False),
        donate_argnums=donate, keep_unused=True)
    sharding = jax.sharding.NamedSharding(mesh, PartitionSpec("core"))
    concat_in = [
        jax.device_put(
            np.concatenate([np.asarray(in_maps[c][n]) for c in range(NCOR)],
                           axis=0), sharding)
        for n in in_names
    ]

    def fresh_zeros():
        return [
            jax.device_put(
                np.zeros((NCOR * z.shape[0], *z.shape[1:]), z.dtype), sharding)
            for z in zero_outs
        ]

    out_arrs = sharded(*concat_in, *fresh_zeros())
    jax.block_until_ready(out_arrs)
    results = [
        {n: np.asarray(out_arrs[i]).reshape(NCOR, *out_avals[i].shape)[c]
         for i, n in enumerate(out_names)}
        for c in range(NCOR)
    ]
    times = []
    for _ in range(iters):
        zs = fresh_zeros()
        jax.block_until_ready(zs)
        t0 = time.perf_counter()
        o = sharded(*concat_in, *zs)
        jax.block_until_ready(o)
        times.append(time.perf_counter() - t0)

    out = np.zeros((cfg.N, cfg.OUT), np.float32)
    for c in range(NCOR):
        op = results[c]["outperm"]
        perm = post["perms"][c]
        out[post["nodes_by_core"][c][perm[:cfg.OWN]]] = op[:cfg.OWN]
    return out, times


def kernel(**inputs):
    cfg = Cfg(N=50000, E=800000)
    out, _ = run(cfg, inputs, trace=False)
    return out


# revision 34
# speedup vs baseline: 1.0352x; 1.0002x over previous
"""2-layer GAT (DGL GATConv-style) on 8 Trainium2 NeuronCores.

Strategy (dst-sharded message passing):
  - Nodes are partitioned across 8 cores round-robin by global in-degree
    rank; core c owns ~6250 dsts and all edges pointing into them.
  - Per-core, own dsts are degree-sorted and tiled 128-at-a-time.  The HBM
    z-table rows are laid out in exactly this (core, tile, slot) order, so
    layer-2 tile results can be written back CONTIGUOUSLY (no scatter), and
    both layers share one set of gather indices.
  - Layer-1 projection z = x @ W1 is computed replicated on every core and
    written to an HBM gather table of 512B bf16 rows.  The per-edge source
    attention term el1[src] is precomputed on the HOST (x @ W1 @ a_l is
    cheap) and staged directly in per-slot layout, so gather rows carry
    only z.
  - Per tile, source rows are fetched with the dma_gather custom DMA
    (int16 indices -> the table is addressed via two overlapping windows
    A=[0,32768) and B=[ROWS-32768,ROWS)).  Edge softmax runs on the
    gathered tile: w = exp(lrelu(el_src + er_dst)), denominators via ACT
    accum, weighting + segment sum on the vector engine (in-place multiply
    + binary-tree folds over the slot axis).
  - Layer-2 projection z2aug = h @ [W2|W2@AL2|W2@AR2] is computed from the
    layer-1 tiles (PE transpose), written contiguously into a per-core
    shard, exchanged with an AllGather collective, then the same
    gather/softmax machinery produces the output.
"""
import sys

sys.path.insert(0, "/opt/trn_rl_repo")

import numpy as np

import concourse.bass as bass
import concourse.mybir as mybir
import concourse.tile as tile_mod
from concourse import library_config
from concourse.library_overlay import lower_extended_insts
from concourse.tile import TileContext
from concourse.bass_utils import run_bass_kernel_spmd

F32 = mybir.dt.float32
BF16 = mybir.dt.bfloat16
I16 = mybir.dt.int16
AF = mybir.ActivationFunctionType
ALU = mybir.AluOpType

NEG_SLOPE = 0.2
SENT_EL = -1.0e30


# ---------------------------------------------------------------------------
# Workaround: this walrus build rejects Drain instructions with >1 sync wait.
def _patched_drain_and_barrier(self, tick_clock, wait_clock):
    nc = self.nc
    probe = nc.sync.drain()
    wait_clock.add_sem_waits(
        probe.ins, tile_mod.ScopedClock({None: tick_clock.global_clock})
    )
    si = probe.ins.sync_info
    waits = list(si.on_wait) if si is not None else []
    if len(waits) > 1:
        bb = nc.cur_bb.bb
        popped = bb.instructions.pop()
        assert popped is probe.ins
        by_name = {}
        for h in self.sems.allocated().values():
            by_name[h.name] = h
        for w in waits:
            assert w.wait_mode == "sem-ge-imm", w
            nc.sync.wait_ge(by_name[w.ant_name], w.wait_value)
        nc.sync.drain()
    nc.all_engine_barrier()
    popped_p = nc._tile_sem_poison_stack.pop()
    assert popped_p is self._sem_poison
    nc.clear_and_free_semaphores(list(self.sems.allocated().values()))
    nc.all_engine_barrier()


TileContext._drain_and_barrier = _patched_drain_and_barrier

_wsplit_n = 0


def _split_multi_waits(nc, keep=1):
    """This walrus build allows at most one sync-wait per instruction; hoist
    extra waits onto dedicated EventSemaphore instructions just before."""
    global _wsplit_n
    for f in nc.m.functions:
        for bb in f.blocks:
            need = any(
                inst.sync_info is not None and len(inst.sync_info.on_wait) > keep
                for inst in bb.instructions
            )
            if not need:
                continue
            newlist = []
            for inst in bb.instructions:
                si = inst.sync_info
                if si is not None and len(si.on_wait) > keep:
                    waits = list(si.on_wait)
                    for w in waits[:-keep]:
                        ev = mybir.InstEventSemaphore(
                            name=f"WSPLIT-{_wsplit_n}", ins=[], outs=[])
                        _wsplit_n += 1
                        ev.engine = inst.engine
                        ev.sync_info = mybir.SyncInfo(on_wait=[w], on_update=[])
                        newlist.append(ev)
                    inst.sync_info = mybir.SyncInfo(
                        on_wait=waits[-keep:], on_update=list(si.on_update))
                newlist.append(inst)
            try:
                bb.instructions[:] = newlist
            except TypeError:
                while len(bb.instructions):
                    bb.instructions.pop()
                for inst in newlist:
                    bb.instructions.append(inst)


# ---------------------------------------------------------------------------
def _pack_idx(logical):
    """int16 idx list -> [32, n/16] wrapped/replicated layout for dma_gather."""
    n = len(logical)
    assert n % 16 == 0
    a = np.asarray(logical, np.int16).reshape(n // 16, 16).T
    out = np.empty((32, n // 16), np.int16)
    out[:16] = a
    out[16:] = a
    return out


class Cfg:
    def __init__(self, N, E, lim=32768):
        self.N = N
        self.E = E
        self.NC = 8
        self.IN = 256
        self.HID = 64
        self.H1 = 4
        self.OUT = 64
        self.OWN = N // self.NC
        self.OWNP = -(-self.OWN // 128) * 128
        self.ROWS = self.NC * self.OWNP
        self.LIM = lim                      # rows addressable by one window
        self.TBOFF = max(self.ROWS - lim, 0)  # start row of window B
        self.NT = self.OWNP // 128          # dst tiles per core
        self.L1C = 256                      # table-1 row (bf16): z only
        self.L2C = 128                      # table-2 row: z2(64) el2 er2 pad
        self.SENT_A = self.OWN              # sentinel row (block 0 pad row)
        self.SENT_B = (self.NC - 1) * self.OWNP + self.OWN
        assert self.SENT_B < self.ROWS
        assert self.SENT_B - self.TBOFF < lim


def prep(cfg, x, W1, al1, ar1, b1, W2, al2, ar2, b2, src, dst):
    """Host-side graph partitioning / staging.  Returns (in_maps, sched, post)."""
    N, E, NC = cfg.N, cfg.E, cfg.NC
    IN, HID, H1, OUT = cfg.IN, cfg.HID, cfg.H1, cfg.OUT

    x = np.asarray(x, np.float32)
    src = np.asarray(src)
    dst = np.asarray(dst)
    W1 = np.asarray(W1, np.float32)
    W2 = np.asarray(W2, np.float32)
    al1 = np.asarray(al1, np.float32)
    ar1 = np.asarray(ar1, np.float32)
    al2 = np.asarray(al2, np.float32)
    ar2 = np.asarray(ar2, np.float32)
    b1 = np.asarray(b1, np.float32)
    b2 = np.asarray(b2, np.float32)

    # parameter transforms
    AL1 = np.zeros((H1 * HID, H1), np.float32)
    AR1 = np.zeros((H1 * HID, H1), np.float32)
    for h in range(H1):
        AL1[h * HID:(h + 1) * HID, h] = al1[h]
        AR1[h * HID:(h + 1) * HID, h] = ar1[h]
    el1 = x @ (W1 @ AL1)                                    # [N, 4] host-side
    er1 = x @ (W1 @ AR1)                                    # [N, 4] host-side
    AL2 = al2.reshape(OUT, 1)
    AR2 = ar2.reshape(OUT, 1)
    W2aug = np.concatenate([W2, W2 @ AL2, W2 @ AR2], axis=1)  # [256, 66]

    bf = mybir.dt.np(BF16)
    # ownership: round-robin by global degree rank -> per-tile max degrees
    # align across cores (shared program, minimal padding)
    deg_g = np.bincount(dst, minlength=N)
    grank = np.argsort(-deg_g, kind="stable")
    owner = np.empty(N, np.int64)
    local_rank = np.empty(N, np.int64)
    owner[grank] = np.arange(N) % NC
    local_rank[grank] = np.arange(N) // NC
    nodes_by_core = [grank[c::NC] for c in range(NC)]   # local-rank order

    # per-core degree-descending dst order; table rows follow this order so
    # layer-2 tile outputs write back contiguously.  Two passes: the second
    # restores the A-window-count tiebreak (which needs provisional rows).
    eids = [np.nonzero(owner[dst] == c)[0] for c in range(NC)]
    dlocs = [local_rank[dst[eids[c]]] for c in range(NC)]
    degs = [np.bincount(dlocs[c], minlength=cfg.OWN) for c in range(NC)]

    def _rows_from(orders):
        pos = []
        for c in range(NC):
            inv = np.empty(cfg.OWN, np.int64)
            inv[orders[c]] = np.arange(cfg.OWN)
            pos.append(inv)
        row = np.empty(N, np.int64)
        for c in range(NC):
            sel = owner == np.int64(c)
            row[sel] = cfg.OWNP * c + pos[c][local_rank[sel]]
        return row

    orders = [np.argsort(-degs[c], kind="stable") for c in range(NC)]
    row_v1 = _rows_from(orders)
    rows_src_v1 = row_v1[src]
    orders = []
    for c in range(NC):
        mA_all = np.bincount(dlocs[c][rows_src_v1[eids[c]] < cfg.TBOFF],
                             minlength=cfg.OWN)
        orders.append(np.lexsort((mA_all, degs[c]))[::-1])
    row_of_node = _rows_from(orders)
    perms = [
        np.concatenate(
            [orders[c], np.full(cfg.OWNP - cfg.OWN, orders[c][-1], np.int64)]
        )
        for c in range(NC)
    ]

    # x laid out in table-row order (pad rows stay zero)
    xrow = np.zeros((cfg.ROWS, IN), bf)
    xrow[row_of_node] = x.astype(bf)
    xT = np.ascontiguousarray(xrow.T)                   # [IN, ROWS] bf16

    rows_src = row_of_node[src]

    # per-core A/B schedule
    tiles_ab, per_core_rows, per_core_nodes = [], [], []
    for c in range(NC):
        eid = np.nonzero(owner[dst] == c)[0]
        dloc = local_rank[dst[eid]]
        srows = rows_src[eid]
        snode = src[eid]
        perm = perms[c]
        so = np.argsort(dloc, kind="stable")
        srows_sorted = srows[so]
        snode_sorted = snode[so]
        starts = np.searchsorted(dloc[so], np.arange(cfg.OWN + 1))
        byd_r = [srows_sorted[starts[i]:starts[i + 1]] for i in range(cfg.OWN)]
        byd_n = [snode_sorted[starts[i]:starts[i + 1]] for i in range(cfg.OWN)]
        per_core_rows.append(byd_r)
        per_core_nodes.append(byd_n)
        ab = []
        for t in range(cfg.NT):
            dts = perm[t * 128:(t + 1) * 128]
            mA = np.zeros(128, np.int64)
            mB = np.zeros(128, np.int64)
            dg = np.zeros(128, np.int64)
            for i in range(128):
                if t * 128 + i >= cfg.OWN:
                    continue
                rs = byd_r[dts[i]]
                dg[i] = len(rs)
                mA[i] = int((rs < cfg.TBOFF).sum())
                mB[i] = int((rs >= cfg.LIM).sum())
            ab.append((mA, mB, dg))
        tiles_ab.append(ab)

    # common per-tile (alpha, beta) across cores, minimizing slot count with
    # dma_gather call count as tiebreak (the per-core beta(alpha) curves are
    # monotone, so one global alpha scan is exact)
    Ks = []
    for t in range(cfg.NT):
        a_lo = max(max(int(tiles_ab[c][t][0].max()) for c in range(NC)), 1)
        a_hi = max(max(int(tiles_ab[c][t][2].max()) for c in range(NC)), 1)
        best, bkey = None, None
        for alpha in range(a_lo, a_hi + 1):
            beta = 1
            for c in range(NC):
                mA, mB, dg = tiles_ab[c][t]
                beta = max(beta, int(
                    np.maximum(mB, dg - np.minimum(alpha, dg - mB)).max()))
            key = ((alpha + beta) * 108 +
                   (-(-alpha // 8) + -(-beta // 8)) * 130)
            if bkey is None or key < bkey:
                best, bkey = (alpha, beta), key
        Ks.append(best)
    EC = 4 * sum(a + b for a, b in Ks)

    # build per-core idx + per-slot el arrays
    idx_all, elb_all, erb_all = [], [], []
    for c in range(NC):
        perm = perms[c]
        byd_r = per_core_rows[c]
        byd_n = per_core_nodes[c]
        cols = []
        elb = np.empty((128, EC), np.float32)
        eloff = 0
        for t in range(cfg.NT):
            a_t, b_t = Ks[t]
            K = a_t + b_t
            Aidx = np.full((a_t, 128), cfg.SENT_A, np.int64)
            Bidx = np.full((b_t, 128), cfg.SENT_B - cfg.TBOFF, np.int64)
            nod = np.full((K, 128), -1, np.int64)
            dts = perm[t * 128:(t + 1) * 128]
            for i in range(128):
                if t * 128 + i >= cfg.OWN:
                    continue
                rs = byd_r[dts[i]]
                ns = byd_n[dts[i]]
                isA = rs < cfg.TBOFF
                isB = rs >= cfg.LIM
                flexm = ~isA & ~isB
                nA = min(a_t, int(isA.sum()) + int(flexm.sum()))
                take = nA - int(isA.sum())
                fidx = np.nonzero(flexm)[0]
                Asel = np.concatenate([np.nonzero(isA)[0], fidx[:take]])
                Bsel = np.concatenate([np.nonzero(isB)[0], fidx[take:]])
                assert len(Asel) <= a_t and len(Bsel) <= b_t, (t, i)
                Aidx[:len(Asel), i] = rs[Asel]
                Bidx[:len(Bsel), i] = rs[Bsel] - cfg.TBOFF
                nod[:len(Asel), i] = ns[Asel]
                nod[a_t:a_t + len(Bsel), i] = ns[Bsel]
            assert Aidx.max() < cfg.LIM
            blkA = _pack_idx(Aidx.reshape(-1))
            blkB = _pack_idx(Bidx.reshape(-1))
            blk = np.concatenate([blkA, blkB], axis=1)
            pad = (-blk.shape[1]) % 32        # keep 64B alignment per tile
            if pad:
                blk = np.concatenate(
                    [blk, np.zeros((32, pad), np.int16)], axis=1)
            cols.append(blk)
            # el block layout [128, 4, K]: head-major, contiguous K per head
            blk = np.full((128, 4, K), SENT_EL, np.float32)
            valid = nod >= 0                                  # [K, 128]
            vi = np.nonzero(valid)
            blk[vi[1][:, None], np.arange(4)[None, :], vi[0][:, None]] = \
                el1[nod[vi]]
            elb[:, eloff:eloff + 4 * K] = blk.reshape(128, 4 * K)
            eloff += 4 * K
        idx_all.append(np.concatenate(cols, axis=1))
        elb_all.append(elb)
        # er1 for own dsts in (tile, partition) layout [128, NT*4]
        own_nodes = nodes_by_core[c][perms[c]]              # [OWNP]
        erb = np.ascontiguousarray(
            er1[own_nodes].reshape(cfg.NT, 128, 4).transpose(1, 0, 2)
            .reshape(128, cfg.NT * 4)).astype(np.float32)
        erb_all.append(erb)

    b1bc = np.broadcast_to(b1.reshape(1, -1), (128, H1 * HID)).copy()
    b2bc = np.broadcast_to(b2.reshape(1, -1), (128, OUT)).copy()
    ident = np.eye(128, dtype=np.float32)
    pad0 = cfg.OWN - (cfg.NT - 1) * 128
    pcap = np.where(np.arange(128) < pad0, 3.0e38, SENT_EL
                    ).astype(np.float32).reshape(128, 1)

    in_maps = []
    for c in range(NC):
        in_maps.append(
            {
                "xT": xT,
                "W1p": W1.astype(bf),
                "erb": erb_all[c],
                "W2aug": W2aug,
                "b1bc": b1bc,
                "b2bc": b2bc,
                "ident": ident,
                "idx_all": idx_all[c],
                "elb": elb_all[c],
                "pcap": pcap,
            }
        )
    sched = {"Ks": Ks, "idx_cols": idx_all[0].shape[1], "EC": EC}
    post = {"perms": perms, "nodes_by_core": nodes_by_core}
    return in_maps, sched, post


# ---------------------------------------------------------------------------
def build(cfg, sched, debug=False, phases=4, g1_mode=5, reps=1, sp=True,
          preload=True, gbufs=6, g2bufs=6):
    Ks = sched["Ks"]
    EC = sched["EC"]
    nc = bass.Bass()
    IN, H1, HID, OUT = cfg.IN, cfg.H1, cfg.HID, cfg.OUT
    L1C, L2C = cfg.L1C, cfg.L2C
    PAD0 = cfg.OWN - (cfg.NT - 1) * 128     # first pad partition of last tile

    def P(name, shape, dt=F32):
        return nc.declare_dram_parameter(name, list(shape), dt, isOutput=False)

    xT = P("xT", [IN, cfg.ROWS], BF16)
    W1p = P("W1p", [IN, 256], BF16)
    erbp = P("erb", [128, cfg.NT * 4])
    W2a = P("W2aug", [IN, 66])
    b1b = P("b1bc", [128, 256])
    b2b = P("b2bc", [128, OUT])
    idn = P("ident", [128, 128])
    idx_all = P("idx_all", [32, sched["idx_cols"]], I16)
    elbp = P("elb", [128, EC])
    pcapp = P("pcap", [128, 1])
    outp = nc.declare_dram_parameter("outperm", [cfg.OWNP, OUT], F32, isOutput=True)

    tab1 = nc.dram_tensor("tab1", [cfg.ROWS, L1C], BF16)
    shard = nc.dram_tensor("shard", [cfg.OWNP, L2C], BF16)
    tab2 = nc.dram_tensor("tab2", [cfg.ROWS, L2C], BF16, addr_space="Shared")

    _regs = {}

    def nreg(v):
        if v not in _regs:
            _regs[v] = nc.gpsimd.to_reg(v)
        return _regs[v]

    with TileContext(nc) as tc:
        nc.gpsimd.load_library(library_config.mlp)
        with tc.tile_pool(name="const", bufs=1) as cp:
            w1a = cp.tile([128, 2 * 256], BF16, tag="w1a")
            w2a = cp.tile([128, 2 * 66], F32, tag="w2a")
            b1s = cp.tile([128, 256], F32, tag="b1s")
            b2s = cp.tile([128, OUT], F32, tag="b2s")
            ids = cp.tile([128, 128], F32, tag="ids")
            er1 = cp.tile([128, cfg.NT * 4], F32, tag="er1")
            er2 = cp.tile([128, cfg.NT], F32, tag="er2")
            els = cp.tile([128, EC], F32, tag="els")
            pcap = cp.tile([128, 1], F32, tag="pcap")
            itall = cp.tile([32, sched["idx_cols"]], I16, tag="itall")
            for k in range(2):
                nc.sync.dma_start(out=w1a[:, k * 256:(k + 1) * 256],
                                  in_=W1p[k * 128:(k + 1) * 128, :])
                nc.sync.dma_start(out=w2a[:, k * 66:(k + 1) * 66],
                                  in_=W2a[k * 128:(k + 1) * 128, :])
            nc.sync.dma_start(out=b1s[:], in_=b1b[:])
            nc.sync.dma_start(out=b2s[:], in_=b2b[:])
            nc.sync.dma_start(out=ids[:], in_=idn[:])
            nc.sync.dma_start(out=els[:], in_=elbp[:])
            nc.sync.dma_start(out=pcap[:], in_=pcapp[:])
            nc.sync.dma_start(out=itall[:], in_=idx_all[:])
            nc.sync.dma_start(out=er1[:], in_=erbp[:])

            for _rep in range(reps):
                # ---------------- phase Z: z table + er1 ----------------
                SUP = 8  # z tiles per x load
                with tc.tile_pool(name="zx", bufs=3) as zxp, \
                     tc.tile_pool(name="zs", bufs=6) as zsp, \
                     tc.tile_pool(name="zp", bufs=4, space="PSUM") as zpp:
                    NRT = cfg.ROWS // 128
                    for st in range(-(-NRT // SUP)):
                        t0 = st * SUP
                        ntl = min(SUP, NRT - t0)
                        cols = ntl * 128
                        xb = zxp.tile([128, 2, cols], BF16, tag="xb")
                        for k in range(2):
                            nc.sync.dma_start(
                                out=xb[:, k, :],
                                in_=xT[k * 128:(k + 1) * 128,
                                       t0 * 128:t0 * 128 + cols])
                        zw = zsp.tile([128, ntl * L1C], BF16, tag="zw")
                        for i in range(ntl):
                            zp_ = zpp.tile([128, 256], F32, tag="zp")
                            for k in range(2):
                                nc.tensor.matmul(
                                    zp_[:], xb[:, k, i * 128:(i + 1) * 128],
                                    w1a[:, k * 256:(k + 1) * 256],
                                    start=(k == 0), stop=(k == 1))
                            nc.scalar.copy(zw[:, i * L1C:(i + 1) * L1C], zp_[:])
                        r0 = t0 * 128
                        zwa = zw[:]
                        dst_ap = bass.AP(
                            tab1, r0 * L1C,
                            [[L1C, 128], [128 * L1C, ntl], [1, L1C]])
                        src_ap = bass.AP(
                            zwa.tensor, zwa.offset,
                            [zwa.ap[0], [L1C, ntl], [1, L1C]])
                        nc.sync.dma_start(out=dst_ap, in_=src_ap)

                # ---------------- phase G1 + T: layer 1 + z2 ----------------
                Kmax = max(a + b for a, b in Ks)
                ioff = 0
                eloff = 0
                with tc.tile_pool(name="g1", bufs=gbufs) as gp, \
                     tc.tile_pool(name="w1p", bufs=3) as wp, \
                     tc.tile_pool(name="ix", bufs=3) as ip, \
                     tc.tile_pool(name="hb", bufs=2) as hp, \
                     tc.tile_pool(name="s2", bufs=2) as s2p, \
                     tc.tile_pool(name="sm", bufs=4) as smp, \
                     tc.tile_pool(name="tp", bufs=2, space="PSUM") as tpp:
                    for t in range(cfg.NT if phases >= 2 else 0):
                        a_t, b_t = Ks[t]
                        K = a_t + b_t
                        icols = -(-K * 8 // 32) * 32
                        if preload:
                            itv = itall[:, ioff:]
                        else:
                            itt = ip.tile([32, K * 8], I16, tag="it")
                            nc.sync.dma_start(
                                out=itt[:], in_=idx_all[:, ioff:ioff + K * 8])
                            itv = itt[:]
                        g = gp.tile([128, Kmax * L1C], BF16, tag="g")
                        gv = g[:, :K * L1C].rearrange("p (k c) -> p k c", c=L1C)
                        # dma_gather crashes the device above ~1024 idxs/instr;
                        # split into <=8-chunk (1024-idx) pieces.
                        for c0 in range(0, a_t, 8):
                            n = min(8, a_t - c0)
                            nc.gpsimd.dma_gather(
                                out_ap=gv[:, c0:c0 + n, :], in_ap=tab1[:],
                                idxs_ap=itv[:, c0 * 8:(c0 + n) * 8],
                                num_idxs=128 * n,
                                num_idxs_reg=nreg(128 * n), elem_size=L1C,
                                single_packet=sp)
                        for c0 in range(0, b_t, 8):
                            n = min(8, b_t - c0)
                            nc.gpsimd.dma_gather(
                                out_ap=gv[:, a_t + c0:a_t + c0 + n, :],
                                in_ap=tab1[cfg.TBOFF:, :],
                                idxs_ap=itv[:, (a_t + c0) * 8:(a_t + c0 + n) * 8],
                                num_idxs=128 * n,
                                num_idxs_reg=nreg(128 * n), elem_size=L1C,
                                single_packet=sp)
                        # scores
                        if g1_mode < 1:
                            ioff += icols
                            eloff += 4 * K
                            continue
                        w = wp.tile([128, Kmax * 4], BF16, tag="w")
                        wv = w[:, :K * 4].rearrange("p (k h) -> p k h", h=4)
                        s = smp.tile([128, 4], F32, tag="s")
                        rs = smp.tile([128, 4], F32, tag="rs")
                        for h in range(4):
                            nc.scalar.activation(
                                wv[:, :, h], els[:, eloff + h * K:eloff + (h + 1) * K],
                                AF.Prelu,
                                bias=er1[:, 4 * t + h:4 * t + h + 1],
                                scale=1.0, alpha=NEG_SLOPE)
                            nc.scalar.activation(
                                wv[:, :, h], wv[:, :, h], AF.Exp,
                                accum_out=s[:, h:h + 1])
                        nc.vector.tensor_scalar_max(s[:], s[:], 1e-30)
                        nc.vector.reciprocal(rs[:], s[:])
                        # weight messages in place ([K, 4, 64] view over rows)
                        ga = g[:]
                        wa = w[:]
                        gz = bass.AP(ga.tensor, ga.offset,
                                     [ga.ap[0], [L1C, K], [HID, 4], [1, HID]])
                        wbc = bass.AP(wa.tensor, wa.offset,
                                      [wa.ap[0], [4, K], [1, 4], [0, HID]])
                        if g1_mode >= 2:
                            nc.vector.tensor_tensor(gz, gz, wbc, op=ALU.mult)
                        # fold over slots
                        Kc = K
                        while Kc > 1 and g1_mode >= 3:
                            half = Kc // 2
                            m = Kc - half
                            lo = bass.AP(ga.tensor, ga.offset,
                                         [ga.ap[0], [L1C, half], [1, 256]])
                            hi = bass.AP(ga.tensor, ga.offset + m * L1C,
                                         [ga.ap[0], [L1C, half], [1, 256]])
                            nc.vector.tensor_tensor(lo, lo, hi, op=ALU.add)
                            Kc = m
                        # epilogue: h = elu(acc * (1/s) + b1)
                        if g1_mode < 4:
                            ioff += icols
                            eloff += 4 * K
                            continue
                        hb = hp.tile([128, 256], F32, tag="hb")
                        acc = bass.AP(ga.tensor, ga.offset,
                                      [ga.ap[0], [HID, 4], [1, HID]])
                        hba = hb[:]
                        hb4 = bass.AP(hba.tensor, hba.offset,
                                      [hba.ap[0], [HID, 4], [1, HID]])
                        rsa = rs[:]
                        rsb = bass.AP(rsa.tensor, rsa.offset,
                                      [rsa.ap[0], [1, 4], [0, HID]])
                        nc.vector.tensor_tensor(hb4, acc, rsb, op=ALU.mult)
                        nc.vector.tensor_tensor(hb[:], hb[:], b1s[:], op=ALU.add)
                        tmp = hp.tile([128, 256], F32, tag="elutmp")
                        nc.vector.tensor_scalar_min(tmp[:], hb[:], 0.0)
                        nc.vector.tensor_scalar_max(hb[:], hb[:], 0.0)
                        nc.scalar.activation(tmp[:], tmp[:], AF.Exp)
                        nc.vector.tensor_tensor(hb[:], hb[:], tmp[:], op=ALU.add)
                        nc.vector.tensor_scalar_add(hb[:], hb[:], -1.0)
                        # transpose + layer-2 projection
                        if g1_mode < 5:
                            ioff += icols
                            eloff += 4 * K
                            continue
                        zp2 = tpp.tile([128, 66], F32, tag="z2p")
                        for k in range(2):
                            tp = tpp.tile([128, 128], F32, tag="tp")
                            nc.tensor.transpose(tp[:], hb[:, k * 128:(k + 1) * 128],
                                                ids[:])
                            hT = s2p.tile([128, 128], F32, tag="hT")
                            nc.scalar.copy(hT[:], tp[:])
                            nc.tensor.matmul(zp2[:], hT[:],
                                             w2a[:, k * 66:(k + 1) * 66],
                                             start=(k == 0), stop=(k == 1))
                        # tab2 row: [z2 64xbf16 | el2 f32-in-2-bf16-slots | pad]
                        z2sb = s2p.tile([128, L2C], BF16, tag="z2sb")
                        nc.scalar.copy(z2sb[:, 0:64], zp2[:, 0:64])
                        z2f = z2sb[:].bitcast(F32)        # [128, 64] f32 view
                        nc.vector.tensor_copy(z2f[:, 32:33], zp2[:, 64:65])
                        nc.vector.tensor_copy(er2[:, t:t + 1], zp2[:, 65:66])
                        if t == cfg.NT - 1:
                            # pad rows: force el2 so layer-2 pad slots weigh 0
                            nc.vector.tensor_tensor(
                                z2f[:, 32:33], z2f[:, 32:33], pcap[:],
                                op=ALU.min)
                        nc.sync.dma_start(
                            out=shard[t * 128:(t + 1) * 128, :], in_=z2sb[:])
                        ioff += icols
                        eloff += 4 * K

                # ---------------- allgather ----------------
                if phases >= 3:
                    nc.gpsimd.collective_compute(
                        "AllGather", ALU.bypass, ins=[shard[:]], outs=[tab2[:]],
                        replica_groups=[list(range(cfg.NC))])

                # ---------------- phase G2: layer 2 ----------------
                ioff = 0
                with tc.tile_pool(name="g2", bufs=g2bufs) as gp2, \
                     tc.tile_pool(name="w2p", bufs=2) as wp2, \
                     tc.tile_pool(name="ix2", bufs=3) as ip2, \
                     tc.tile_pool(name="ob", bufs=2) as op_, \
                     tc.tile_pool(name="sm2", bufs=4) as smp2:
                    for t in range(cfg.NT if phases >= 4 else 0):
                        a_t, b_t = Ks[t]
                        K = a_t + b_t
                        icols = -(-K * 8 // 32) * 32
                        if preload:
                            itv = itall[:, ioff:]
                        else:
                            itt = ip2.tile([32, K * 8], I16, tag="it2")
                            nc.sync.dma_start(
                                out=itt[:], in_=idx_all[:, ioff:ioff + K * 8])
                            itv = itt[:]
                        g = gp2.tile([128, Kmax * L2C], BF16, tag="g2")
                        gv = g[:, :K * L2C].rearrange("p (k c) -> p k c", c=L2C)
                        for c0 in range(0, a_t, 8):
                            n = min(8, a_t - c0)
                            nc.gpsimd.dma_gather(
                                out_ap=gv[:, c0:c0 + n, :], in_ap=tab2[:],
                                idxs_ap=itv[:, c0 * 8:(c0 + n) * 8],
                                num_idxs=128 * n,
                                num_idxs_reg=nreg(128 * n), elem_size=L2C,
                                single_packet=sp)
                        for c0 in range(0, b_t, 8):
                            n = min(8, b_t - c0)
                            nc.gpsimd.dma_gather(
                                out_ap=gv[:, a_t + c0:a_t + c0 + n, :],
                                in_ap=tab2[cfg.TBOFF:, :],
                                idxs_ap=itv[:, (a_t + c0) * 8:(a_t + c0 + n) * 8],
                                num_idxs=128 * n,
                                num_idxs_reg=nreg(128 * n), elem_size=L2C,
                                single_packet=sp)
                        w2t = wp2.tile([128, Kmax], F32, tag="w2t")
                        s2 = smp2.tile([128, 1], F32, tag="s2")
                        rs2 = smp2.tile([128, 1], F32, tag="rs2")
                        ga = g[:]
                        gf = g[:].bitcast(F32)            # [128, Kmax*64]
                        el2 = bass.AP(gf.tensor, gf.offset + 32,
                                      [gf.ap[0], [L2C // 2, K]])
                        nc.scalar.activation(
                            w2t[:, :K], el2, AF.Prelu,
                            bias=er2[:, t:t + 1], scale=1.0, alpha=NEG_SLOPE)
                        nc.scalar.activation(
                            w2t[:, :K], w2t[:, :K], AF.Exp, accum_out=s2[:])
                        nc.vector.tensor_scalar_max(s2[:], s2[:], 1e-30)
                        nc.vector.reciprocal(rs2[:], s2[:])
                        # weight into f32 accumulator, then fold in f32
                        gw = wp2.tile([128, Kmax * OUT], F32, tag="gw")
                        gwa = gw[:]
                        wa = w2t[:]
                        gz = bass.AP(ga.tensor, ga.offset,
                                     [ga.ap[0], [L2C, K], [1, OUT]])
                        gwz = bass.AP(gwa.tensor, gwa.offset,
                                      [gwa.ap[0], [OUT, K], [1, OUT]])
                        wbc = bass.AP(wa.tensor, wa.offset,
                                      [wa.ap[0], [1, K], [0, OUT]])
                        nc.vector.tensor_tensor(gwz, gz, wbc, op=ALU.mult)
                        Kc = K
                        while Kc > 1:
                            half = Kc // 2
                            m = Kc - half
                            lo = bass.AP(gwa.tensor, gwa.offset,
                                         [gwa.ap[0], [OUT, half], [1, OUT]])
                            hi = bass.AP(gwa.tensor, gwa.offset + m * OUT,
                                         [gwa.ap[0], [OUT, half], [1, OUT]])
                            nc.vector.tensor_tensor(lo, lo, hi, op=ALU.add)
                            Kc = m
                        ob = op_.tile([128, OUT], F32, tag="ob")
                        nc.vector.tensor_scalar_mul(ob[:], gw[:, 0:OUT], rs2[:])
                        nc.vector.tensor_tensor(ob[:], ob[:], b2s[:], op=ALU.add)
                        nc.sync.dma_start(
                            out=outp[t * 128:(t + 1) * 128, :], in_=ob[:])
                        ioff += icols

    _split_multi_waits(nc)
    lower_extended_insts(nc)
    return nc


# ---------------------------------------------------------------------------
_memo = {}


def run(cfg, inputs, trace=False, debug=False):
    in_maps, sched, post = prep(cfg, **inputs)
    key = (cfg.N, cfg.E, cfg.LIM, tuple(sched["Ks"]), bool(debug))
    if key not in _memo:
        _memo[key] = build(cfg, sched, debug=debug)
    nc = _memo[key]
    res = run_bass_kernel_spmd(
        nc, in_maps, list(range(cfg.NC)), trace=trace)
    out = np.zeros((cfg.N, cfg.OUT), np.float32)
    for c in range(cfg.NC):
        op = res.results[c]["outperm"]
        perm = post["perms"][c]
        out[post["nodes_by_core"][c][perm[:cfg.OWN]]] = op[:cfg.OWN]
    return out, res


def run_bench(cfg, inputs, iters=3, reps=1):
    """Run once for outputs, then time repeated executions of the compiled
    NEFF (inputs pre-staged on device, outputs donated fresh each iter).
    reps repeats the whole pipeline inside one NEFF; timing two different
    reps values isolates per-iteration device time from launch latency."""
    import time

    import jax
    from jax.experimental.shard_map import shard_map
    from jax.sharding import Mesh, PartitionSpec

    from concourse import bass2jax

    bass2jax.install_neuronx_cc_hook()

    in_maps, sched, post = prep(cfg, **inputs)
    key = (cfg.N, cfg.E, cfg.LIM, tuple(sched["Ks"]), False, reps)
    if key not in _memo:
        _memo[key] = build(cfg, sched, reps=reps)
    nc = _memo[key]

    partition_name = nc.partition_id_tensor.name if nc.partition_id_tensor else None
    in_names, out_names, out_avals, zero_outs = [], [], [], []
    for alloc in nc.m.functions[0].allocations:
        if not isinstance(alloc, mybir.MemoryLocationSet):
            continue
        name = alloc.memorylocations[0].name
        if alloc.kind == "ExternalInput":
            if name != partition_name:
                in_names.append(name)
        elif alloc.kind == "ExternalOutput":
            out_names.append(name)
            shape = tuple(alloc.tensor_shape)
            dtype = mybir.dt.np(alloc.dtype)
            out_avals.append(jax.core.ShapedArray(shape, dtype))
            zero_outs.append(np.zeros(shape, dtype))
    n_params = len(in_names)
    n_outs = len(out_avals)
    all_in_names = list(in_names) + list(out_names)
    if partition_name is not None:
        all_in_names.append(partition_name)
    donate = tuple(range(n_params, n_params + n_outs))

    def _body(*args):
        operands = list(args)
        if partition_name is not None:
            operands.append(bass2jax.partition_id_tensor())
        outs = bass2jax._bass_exec_p.bind(
            *operands,
            out_avals=tuple(out_avals),
            in_names=tuple(all_in_names),
            out_names=tuple(out_names),
            lowering_input_output_aliases=(),
            sim_require_finite=True,
            sim_require_nnan=True,
            nc=nc,
        )
        return tuple(outs)

    NCOR = cfg.NC
    devices = jax.devices()[:NCOR]
    mesh = Mesh(np.asarray(devices), ("core",))
    in_specs = (PartitionSpec("core"),) * (n_params + n_outs)
    out_specs = (PartitionSpec("core"),) * len(out_names)
    sharded = jax.jit(
        shard_map(_body, mesh=mesh, in_specs=in_specs, out_specs=out_specs,
                  check_rep=False),
        donate_argnums=donate, keep_unused=True)
    sharding = jax.sharding.NamedSharding(mesh, PartitionSpec("core"))
    concat_in = [
        jax.device_put(
            np.concatenate([np.asarray(in_maps[c][n]) for c in range(NCOR)],
                           axis=0), sharding)
        for n in in_names
    ]

    def fresh_zeros():
        return [
            jax.device_put(
                np.zeros((NCOR * z.shape[0], *z.shape[1:]), z.dtype), sharding)
            for z in zero_outs
        ]

    out_arrs = sharded(*concat_in, *fresh_zeros())
    jax.block_until_ready(out_arrs)
    results = [
        {n: np.asarray(out_arrs[i]).reshape(NCOR, *out_avals[i].shape)[c]
         for i, n in enumerate(out_names)}
        for c in range(NCOR)
    ]
    times = []
    for _ in range(iters):
        zs = fresh_zeros()
        jax.block_until_ready(zs)
        t0 = time.perf_counter()
        o = sharded(*concat_in, *zs)
        jax.block_until_ready(o)
        times.append(time.perf_counter() - t0)

    out = np.zeros((cfg.N, cfg.OUT), np.float32)
    for c in range(NCOR):
        op = results[c]["outperm"]
        perm = post["perms"][c]
        out[post["nodes_by_core"][c][perm[:cfg.OWN]]] = op[:cfg.OWN]
    return out, times


def kernel(**inputs):
    cfg = Cfg(N=50000, E=800000)
    out, _ = run(cfg, inputs, trace=False)
    return out
